# revision 13
# baseline (speedup 1.0000x reference)
"""Trainium2 Bass kernel for the dendritic-branch spiking FNN (DH_SFNN).

Model (per reference):
  branch_in = x @ W_in.T + b_in                  # (B,T,H*BR)
  per t:  i_d = beta*i_d + (1-beta)*branch_in_t  # beta = sigmoid(tau_n), (H,BR)
          v   = alpha*v + (1-alpha)*i_d.sum(br)  # alpha = sigmoid(tau_m), (H,)
          spike = (v >= 1); v -= spike; counts += spike
  out = counts @ W_out.T + b_out                 # (B,D_OUT)

Strategy: data-parallel over batch across 8 cores (32 rows each). Per core,
T=500 is processed in chunks pipelined across engines.

GEMM: fp16 "hh" pass (W_hi @ x_hi, 6 k-tiles) into PSUM P0, plus fp8e4m3
DoubleRow correction products into PSUM P1. The corrections recover the
fp16-split residuals (W_lo @ x_hi + W_hi @ x_lo) from power-of-2-scaled fp8
operands; DoubleRow runs fp8 matmuls at 0.5 cycles/row with 256-deep
contraction, so each correction product costs 1/4 of an fp16 pass. The W
operands are UNSCALED (uniform row magnitudes) -- folding the per-row
(1-beta)(1-alpha) scale into W pushes the fp8 splits into subnormal flush and
fails the accuracy gate; instead sc2 stays in the Act epilogue:
  Act#1: u0 = sc2*P0 + b2   (baseline epilogue)
  Act#2: c  = (sc2*2^-17)*P1   (per-partition scale AP)
  Pool:  u  = u0 + c
Host emulation: 2-product corrections reproduce the fp32 reference's spike
train exactly on this data (rel 1e-7, same as the fp16 3-pass baseline).

Engines:
  PE  : hh fp16 matmuls + fp8 DR corrections + readout
  Act : PSUM epilogues (two per (m,g)), Sign() spike-compare on hist
  Pool: epilogue combine adds, branch-sum adds
  DVE : IIR scans, carry handling, serial per-timestep spike loop,
        spike-count time reduction
"""

import sys

if "/opt/trn_rl_repo" not in sys.path:
    sys.path.insert(0, "/opt/trn_rl_repo")

from contextlib import ExitStack

import numpy as np
import ml_dtypes

import concourse.bass as bass
import concourse.mybir as mybir
import concourse.tile as tile
from concourse import bacc

B, T, D_IN, H, BR, D_OUT = 256, 500, 700, 200, 2, 35
NCORES = 8
BL = B // NCORES          # local batch = 32
NK = 6                    # k-tiles; D_IN padded 700 -> 768 so every tile is 128
DP = NK * 128             # padded contraction dim (768)
M = 4                     # m-tiles, m=(br,j): o'' = m*128 + p, h = (m%2)*128+p
OP = M * 128              # padded output rows (512)
NJ = 2                    # h groups (j=0: h<128, j=1: h 128..199)
NF = NJ * BL              # spike-loop state columns (64)
BG = 8                    # batches per matmul n-group
NG = BL // BG             # 4 n-groups

CHUNKS = (50,) * 9 + (25, 15, 10)     # sum = T; shrinking tail chunks

# fp8 correction scheme: NPROD products (wvar, xvar); all products land at
# scale PSCALE = 2^16 in the SAME PSUM bank as the fp16 hh pass, whose weights
# are pre-scaled by 2^16 (exact power-of-2 in fp16; max |W|*2^16 ~ 13K < 65504).
# The single Act epilogue applies sc2*2^-16.
SXA, SWH = 2.0 ** 11, 2.0 ** 5        # x_lo*SXA, W_hi*SWH
SXH, SWL = 2.0 ** 4, 2.0 ** 12        # x_hi*SXH, W_lo*SWL
PSCALE = 2.0 ** 16
E4M3 = ml_dtypes.float8_e4m3

# product tables by nprod: (w split index, x split index)
# w splits: 0=wh1, 1=wh2, 2=wa, 3=wb ; x splits: 0=xa, 1=xb, 2=xh1, 3=xh2
PRODUCTS = {
    2: [(0, 0), (2, 2)],
    6: [(0, 0), (0, 1), (1, 0), (2, 2), (3, 2), (2, 3)],
}
XVARS = {2: [0, 2], 6: [0, 1, 2, 3]}  # distinct x splits shipped


def _f32(a):
    return np.ascontiguousarray(a, dtype=np.float32)


def _build(T_, chunks, nprod, alpha_uniform_val=None):
    chunks = tuple(chunks)
    NCH = len(chunks)
    assert sum(chunks) == T_
    C0 = max(chunks)
    sizes = sorted(set(chunks))
    prods = PRODUCTS[nprod]
    xvars = XVARS[nprod]
    NXV = len(xvars)
    xv_pos = {v: i for i, v in enumerate(xvars)}   # x split -> slot in x8
    wlist = sorted({p[0] for p in prods})          # distinct w splits shipped
    w_pos = {v: i for i, v in enumerate(wlist)}
    NW = len(wlist)

    # x columns per (chunk, group) and offsets in the flat tensors
    xoff16 = np.cumsum([0] + [NK * BG * cc for cc in chunks]).tolist()
    FT16 = xoff16[-1]
    xoff8 = np.cumsum([0] + [NK * NXV * BG * cc for cc in chunks]).tolist()
    FT8 = xoff8[-1]
    # d0 blocks per distinct chunk size
    soff = {}
    off = 0
    for s in sizes:
        soff[s] = off
        off += M * BL * s
    SD = off

    fp32 = mybir.dt.float32
    fp16 = mybir.dt.float16
    fp8 = mybir.dt.float8e4
    AF = mybir.ActivationFunctionType
    AL = mybir.AluOpType
    PM = mybir.MatmulPerfMode

    nc = bacc.Bacc("TRN2", target_bir_lowering=False, debug=False,
                   num_devices=NCORES)

    xt_d = nc.dram_tensor("xt", [NG, 128, FT16], fp16, kind="ExternalInput")
    x8_d = nc.dram_tensor("x8", [NG, 128, FT8], fp8, kind="ExternalInput")
    wt_d = nc.dram_tensor("wt", [NK, 128, OP], fp16, kind="ExternalInput")
    w8_d = nc.dram_tensor("w8", [NW, NK, 128, OP], fp8, kind="ExternalInput")
    sc2_d = nc.dram_tensor("sc2", [128, M], fp32, kind="ExternalInput")
    sc28_d = nc.dram_tensor("sc28", [128, M], fp32, kind="ExternalInput")
    b2_d = nc.dram_tensor("b2", [128, M], fp32, kind="ExternalInput")
    bt_d = nc.dram_tensor("bt", [128, M], fp32, kind="ExternalInput")
    atile_d = nc.dram_tensor("atile", [128, NF], fp32, kind="ExternalInput")
    woutT_d = nc.dram_tensor("woutT", [2 * 128, D_OUT], fp32, kind="ExternalInput")
    bout_d = nc.dram_tensor("bout", [D_OUT, 1], fp32, kind="ExternalInput")

    out_d = nc.dram_tensor("out", [D_OUT, BL], fp32, kind="ExternalOutput")
    tok_d = nc.dram_tensor("tok", [1, 16], fp32, kind="ExternalInput")
    tok_o = nc.dram_tensor("tok_out", [1, 16], fp32, kind="ExternalOutput")

    with tile.TileContext(nc) as tc, ExitStack() as ctx:
        const = ctx.enter_context(tc.tile_pool(name="const", bufs=1))
        st = ctx.enter_context(tc.tile_pool(name="state", bufs=1))
        up = ctx.enter_context(tc.tile_pool(name="up", bufs=2))
        wp = ctx.enter_context(tc.tile_pool(name="wph", bufs=2))
        hp = ctx.enter_context(tc.tile_pool(name="hp", bufs=2))
        xp = ctx.enter_context(tc.tile_pool(name="xin", bufs=3))
        ps = ctx.enter_context(tc.tile_pool(name="psum", bufs=2, space="PSUM"))
        pso = ctx.enter_context(tc.tile_pool(name="psout", bufs=1, space="PSUM"))
        scr = ctx.enter_context(tc.tile_pool(name="scr", bufs=2))

        w_sb = const.tile([128, NK * OP], fp16, tag="wsb")
        nc.sync.dma_start(
            w_sb[:].rearrange("p (k o) -> p k o", k=NK),
            wt_d.ap().rearrange("k p o -> p k o"))
        w8_sb = const.tile([128, NW * NK * OP], fp8, tag="w8sb")
        nc.scalar.dma_start(
            w8_sb[:].rearrange("p (q k o) -> p q k o", q=NW, k=NK),
            w8_d.ap().rearrange("q k p o -> p q k o"))
        sc2 = const.tile([128, M], fp32)
        nc.sync.dma_start(sc2[:], sc2_d.ap())
        sc28 = const.tile([128, M], fp32)
        nc.sync.dma_start(sc28[:], sc28_d.ap())
        b2 = const.tile([128, M], fp32)
        nc.sync.dma_start(b2[:], b2_d.ap())
        bt = const.tile([128, M], fp32)
        nc.scalar.dma_start(bt[:], bt_d.ap())
        atile = const.tile([128, NF], fp32)
        nc.scalar.dma_start(atile[:], atile_d.ap())
        # d0 (scan multipliers: beta everywhere, 0 at each batch's t=0)
        d0_sb = const.tile([128, SD], fp32)
        nc.vector.memset(d0_sb[:], 0.0)
        for si, s in enumerate(sizes):
            for m in range(M):
                blk = d0_sb[:, soff[s] + m * BL * s:
                            soff[s] + (m + 1) * BL * s]
                nc.scalar.activation(blk, blk, AF.Identity,
                                     bias=bt[:, m:m + 1], scale=0.0)
                nc.vector.memset(
                    blk.rearrange("p (b c) -> p b c", c=s)[:, :, 0], 0.0)
        woutT_sb = const.tile([128, 2 * D_OUT], fp32)
        nc.scalar.dma_start(woutT_sb[:, 0:D_OUT], woutT_d.ap()[0:128])
        nc.scalar.dma_start(woutT_sb[:, D_OUT:2 * D_OUT], woutT_d.ap()[128:256])
        bout_sb = const.tile([D_OUT, 1], fp32)
        nc.scalar.dma_start(bout_sb[:], bout_d.ap())
        negone = const.tile([128, 1], fp32, tag="negone")
        nc.vector.memset(negone[:], -1.0)

        w8v = w8_sb[:].rearrange("p (q k o) -> p q k o", q=NW, k=NK)

        def cmp_phase(hist, cc):
            hv = hist[:, 0:cc * NF]
            nc.scalar.activation(hv, hv, AF.Sign,
                                 bias=negone[:, 0:1], scale=negone[:, 0:1])

        def red_phase(hist, cc, counts):
            # add-tree on Pool over the t-major hist: fold halves until one
            # NF-wide column block remains, then accumulate into counts.
            n = cc
            base = 0

            def blk(i0, ln):
                return hist[:, (base + i0) * NF:(base + i0 + ln) * NF]

            while n > 1:
                h = n // 2
                nc.gpsimd.tensor_tensor(blk(0, h), blk(0, h), blk(h, h),
                                        AL.add)
                if n % 2:
                    nc.gpsimd.tensor_tensor(blk(0, 1), blk(0, 1),
                                            blk(n - 1, 1), AL.add)
                n = h
            nc.gpsimd.tensor_tensor(counts[:], counts[:], blk(0, 1), AL.add)

        def body_once():
            vst = st.tile([128, NF], fp32, tag="vst")  # negated potential
            counts = st.tile([128, NF], fp32, tag="cnt")
            carry = st.tile([128, M * BL], fp32, tag="carry")
            nc.vector.memset(vst[:], 0.0)
            nc.vector.memset(counts[:], 0.0)

            prev = None     # (hist, chunk_len) of previous chunk
            for c, CC in enumerate(chunks):
                NNc = BG * CC
                u = up.tile([128, M * BL * C0], fp32, tag="u")
                wti = wp.tile([128, C0 * NF], fp32, tag="wti")
                hist = hp.tile([128, C0 * NF], fp32, tag="hist")

                # -- GEMM: u[m-tile, b, t] = sc28*(x @ W') + b2 --
                # n-groups processed in pairs sharing each weight load: the
                # stationary tensor is loaded once per (k, m) and streamed
                # over both groups' moving data.
                ndr = len(prods) * (NK // 2)
                for gp in range(NG // 2):
                    xs16 = []
                    xs8 = []
                    for gi in range(2):
                        g = 2 * gp + gi
                        x16 = xp.tile([128, NK * BG * C0], fp16, tag="x16")
                        nc.sync.dma_start(
                            x16[:, 0:NK * NNc],
                            xt_d.ap()[g][:, xoff16[c]:xoff16[c + 1]])
                        x8 = xp.tile([128, NK * NXV * BG * C0], fp8, tag="x8")
                        nc.sync.dma_start(
                            x8[:, 0:NK * NXV * NNc],
                            x8_d.ap()[g][:, xoff8[c]:xoff8[c + 1]])
                        xs16.append(x16)
                        xs8.append(x8[:, 0:NK * NXV * NNc].rearrange(
                            "p (k v n) -> p k v n", k=NK, v=NXV))
                    for m in range(M):
                        pts = [ps.tile([128, NNc], fp32, tag=f"pt{gi}",
                                       name=f"pt{gi}") for gi in range(2)]
                        for k in range(NK):
                            wap = w_sb[:, k * OP + m * 128:
                                       k * OP + (m + 1) * 128]
                            for gi in range(2):
                                nc.tensor.matmul(
                                    pts[gi][:], wap,
                                    xs16[gi][:, k * NNc:(k + 1) * NNc],
                                    start=(k == 0), stop=False)
                        i = 0
                        for (wv, xv) in prods:
                            for j in range(NK // 2):
                                w8ap = w8v[:, w_pos[wv], 2 * j:2 * j + 2,
                                           m * 128:(m + 1) * 128]
                                for gi in range(2):
                                    nc.tensor.matmul(
                                        pts[gi][:], w8ap,
                                        xs8[gi][:, 2 * j:2 * j + 2,
                                                xv_pos[xv]],
                                        start=False, stop=(i == ndr - 1),
                                        perf_mode=PM.DoubleRow)
                                i += 1
                        for gi in range(2):
                            g = 2 * gp + gi
                            nc.scalar.activation(
                                u[:, m * BL * C0 + g * NNc:
                                  m * BL * C0 + (g + 1) * NNc],
                                pts[gi][:], AF.Identity,
                                bias=b2[:, m:m + 1], scale=sc28[:, m:m + 1])

                # spike-compare of previous chunk on Act
                if prev is not None:
                    cmp_phase(*prev)

                # -- dendrite IIR: i_d = beta*i_d + u, fused scan per m-tile --
                for m in range(M):
                    um = u[:, m * BL * C0:m * BL * C0 + BL * CC]
                    um3 = um.rearrange("p (b c) -> p b c", c=CC)
                    if c > 0:
                        nc.vector.scalar_tensor_tensor(
                            um3[:, :, 0], carry[:, m * BL:(m + 1) * BL],
                            bt[:, m:m + 1], um3[:, :, 0], AL.mult, AL.add)
                    d0c = soff[CC] + m * BL * CC
                    nc.vector.tensor_tensor_scan(
                        um[:], d0_sb[:, d0c:d0c + BL * CC],
                        um[:], 0.0, AL.mult, AL.add)
                    if c < NCH - 1:
                        nc.vector.tensor_copy(carry[:, m * BL:(m + 1) * BL],
                                              um3[:, :, CC - 1])

                # count-reduce of previous chunk
                if prev is not None:
                    red_phase(prev[0], prev[1], counts)

                # -- branch sum: w_j = i'_d[j] + i'_d[2+j] --
                wre = wti[:, 0:CC * NF].rearrange("p (c j b) -> p b j c",
                                                  j=NJ, b=BL)
                for j in range(NJ):
                    nc.gpsimd.tensor_tensor(
                        wre[:, :, j, :],
                        u[:, j * BL * C0:j * BL * C0 + BL * CC].rearrange(
                            "p (b c) -> p b c", c=CC),
                        u[:, (2 + j) * BL * C0:(2 + j) * BL * C0 + BL * CC
                          ].rearrange("p (b c) -> p b c", c=CC), AL.add)

                # -- spike loop (negated state: vt = -v) --
                for t in range(CC):
                    tA = hist[:, t * NF:(t + 1) * NF]   # pre-reset vt' kept
                    wt_t = wti[:, t * NF:(t + 1) * NF]
                    if alpha_uniform_val is not None:
                        nc.vector.scalar_tensor_tensor(
                            tA, vst[:], float(alpha_uniform_val),
                            wt_t, AL.mult, AL.subtract)
                    else:
                        nc.vector.tensor_tensor(tA, vst[:], atile[:], AL.mult)
                        nc.vector.tensor_tensor(tA, tA, wt_t, AL.subtract)
                    nc.vector.scalar_tensor_tensor(
                        vst[:], tA, -1.0, tA, AL.is_le, AL.add)
                prev = (hist, CC)

            # final chunk: compare inline on DVE
            fh, fcc = prev
            nc.vector.tensor_scalar(fh[:, 0:fcc * NF], fh[:, 0:fcc * NF],
                                    -1.0, None, AL.is_le)
            csc = scr.tile([128, NF], fp32, tag="csc")
            nc.vector.tensor_reduce(
                csc[:], fh[:, 0:fcc * NF].rearrange("p (c f) -> p f c", f=NF),
                mybir.AxisListType.X, AL.add)
            nc.vector.scalar_tensor_tensor(
                counts[:], csc[:], 2.0, counts[:], AL.mult, AL.add)

            # -- readout --
            po = pso.tile([D_OUT, BL], fp32, tag="po")
            nc.tensor.matmul(po[:], woutT_sb[:, 0:D_OUT], counts[:, 0:BL],
                             start=True, stop=False)
            nc.tensor.matmul(po[:], woutT_sb[0:H - 128, D_OUT:2 * D_OUT],
                             counts[0:H - 128, BL:2 * BL], start=False,
                             stop=True)
            out_sb = scr.tile([D_OUT, BL], fp32, tag="osb")
            nc.scalar.activation(out_sb[:], po[:], AF.Identity,
                                 bias=bout_sb[:, 0:1], scale=1.0)
            nc.sync.dma_start(out_d.ap(), out_sb[:])

        body_once()
        tok_sb = scr.tile([1, 16], fp32, tag="tok")
        nc.sync.dma_start(tok_sb[:], tok_d.ap())
        nc.sync.dma_start(tok_o.ap(), tok_sb[:])

    nc.compile()
    return nc


def _prep_host(x, W_in, b_in, tau_n, tau_m, W_out, b_out, T_, chunks, nprod):
    """Host-side constant prep. Returns (shared_inputs, per_core_inputs, alpha_uni)."""
    x = _f32(x); W_in = _f32(W_in); b_in = _f32(b_in)
    tau_n = _f32(tau_n); tau_m = _f32(tau_m)
    W_out = _f32(W_out); b_out = _f32(b_out)
    chunks = tuple(chunks)
    assert sum(chunks) == T_
    prods = PRODUCTS[nprod]
    xvars = XVARS[nprod]
    NXV = len(xvars)
    NW = len({p[0] for p in prods})

    beta = _f32(1.0 / (1.0 + np.exp(-tau_n.astype(np.float64))))   # (H,BR)
    alpha = _f32(1.0 / (1.0 + np.exp(-tau_m.astype(np.float64))))  # (H,)
    one = np.float32(1.0)

    def fp8q(a):
        return np.ascontiguousarray(a, dtype=np.float32).astype(E4M3)

    # m-tile map: m=(br,j) -> rows p: h = (m%2)*128+p, o = h*BR + br
    wt = np.zeros((NK, 128, OP), np.float32)
    sc2 = np.zeros((128, M), np.float32)
    sc28 = np.zeros((128, M), np.float32)
    b2 = np.zeros((128, M), np.float32)
    bt = np.zeros((128, M), np.float32)
    for m in range(M):
        br, j = m // 2, m % 2
        for p in range(128):
            h = j * 128 + p
            if h >= H:
                continue
            o = h * BR + br
            s = (one - beta[h, br]) * (one - alpha[h])
            sc2[p, m] = s
            sc28[p, m] = s * np.float32(1.0 / PSCALE)
            b2[p, m] = s * b_in[o]
            bt[p, m] = beta[h, br]
            wrow = np.zeros(DP, np.float32)
            wrow[:D_IN] = W_in[o]
            wt[:, :, m * 128 + p] = wrow.reshape(NK, 128)
    wh = wt.astype(np.float16).astype(np.float32)
    wl = wt - wh
    # w splits: 0=wh1, 1=wh2, 2=wa, 3=wb (scaled, stored fp8)
    wh1 = fp8q(wh * np.float32(SWH))
    wh2 = fp8q(wh * np.float32(SWH) - wh1.astype(np.float32))
    wa = fp8q(wl * np.float32(SWL))
    wb = fp8q(wl * np.float32(SWL) - wa.astype(np.float32))
    wsplit_all = [wh1, wh2, wa, wb]
    w8 = np.stack([wsplit_all[i] for i in sorted({p[0] for p in prods})])

    atile = np.zeros((128, NF), np.float32)
    for j in range(NJ):
        for p in range(128):
            h = j * 128 + p
            if h >= H:
                continue
            atile[p, j * BL:(j + 1) * BL] = alpha[h]
    woutT = np.zeros((256, D_OUT), np.float32)
    woutT[:H, :] = 0.5 * W_out.T
    teff = np.float32(T_ - chunks[-1])
    bout = (b_out + 0.5 * teff * W_out.sum(axis=1)).reshape(D_OUT, 1)

    shared = dict(wt=(wt * np.float32(PSCALE)).astype(np.float16),
                  w8=w8.view(np.uint8),
                  sc2=sc2, sc28=sc28, b2=b2, bt=bt, atile=atile,
                  woutT=_f32(woutT), bout=_f32(bout))

    percore = []
    for core in range(NCORES):
        xl_ = x[core * BL:(core + 1) * BL, :T_, :]        # (BL,T,D_IN)
        xp_ = np.zeros((BL, T_, DP), np.float32)
        xp_[:, :, :D_IN] = xl_
        xh = xp_.astype(np.float16).astype(np.float32)
        xlo = xp_ - xh
        # x splits: 0=xa, 1=xb, 2=xh1, 3=xh2
        xa = fp8q(xlo * np.float32(SXA))
        xsplit = {0: xa}
        if 1 in xvars:
            xsplit[1] = fp8q(xlo * np.float32(SXA) - xa.astype(np.float32))
        xh1 = fp8q(xh * np.float32(SXH))
        xsplit[2] = xh1
        if 3 in xvars:
            xsplit[3] = fp8q(xh * np.float32(SXH) - xh1.astype(np.float32))

        FT16 = sum(NK * BG * cc for cc in chunks)
        FT8 = sum(NK * NXV * BG * cc for cc in chunks)
        xt = np.zeros((NG, 128, FT16), np.float16)
        x8 = np.zeros((NG, 128, FT8), E4M3)
        colo16 = 0
        colo8 = 0
        t0 = 0
        xh16 = xp_.astype(np.float16)
        for cc in chunks:
            for g in range(NG):
                sub = xh16[g * BG:(g + 1) * BG, t0:t0 + cc, :]   # (BG,cc,DP)
                sg = sub.reshape(BG, cc, NK, 128).transpose(3, 2, 0, 1)
                xt[g, :, colo16:colo16 + NK * BG * cc] = sg.reshape(128, -1)
                # x8 layout: (k, v, b, t)
                blk = np.empty((128, NK, NXV, BG, cc), E4M3)
                for vi, v in enumerate(xvars):
                    sv = xsplit[v][g * BG:(g + 1) * BG, t0:t0 + cc, :]
                    blk[:, :, vi] = sv.reshape(BG, cc, NK, 128).transpose(
                        3, 2, 0, 1)
                x8[g, :, colo8:colo8 + NK * NXV * BG * cc] = blk.reshape(128, -1)
            colo16 += NK * BG * cc
            colo8 += NK * NXV * BG * cc
            t0 += cc
        percore.append(dict(xt=xt, x8=x8.view(np.uint8)))
    uni = float(alpha[0]) if np.all(alpha == alpha[0]) else None
    return shared, percore, uni


TRACE = False          # set by test harness for profiling runs
LAST_RESULT = None
NPROD = 2


def kernel(x, W_in, b_in, tau_n, tau_m, W_out, b_out):
    global LAST_RESULT
    from concourse.bass_utils import run_bass_kernel_spmd

    shared, percore, uni = _prep_host(x, W_in, b_in, tau_n, tau_m, W_out,
                                      b_out, T, CHUNKS, NPROD)
    nc = _build(T, CHUNKS, NPROD, alpha_uniform_val=uni)
    tok = np.zeros((1, 16), np.float32)
    in_maps = [dict(shared, tok=tok, **percore[core])
               for core in range(NCORES)]
    res = run_bass_kernel_spmd(nc, in_maps, core_ids=list(range(NCORES)),
                               trace=TRACE)
    LAST_RESULT = res
    out = np.empty((B, D_OUT), np.float32)
    for core in range(NCORES):
        out[core * BL:(core + 1) * BL, :] = res.results[core]["out"].T
    return out


# revision 15
# speedup vs baseline: 1.0227x; 1.0227x over previous
"""Trainium2 Bass kernel for the dendritic-branch spiking FNN (DH_SFNN).

Model (per reference):
  branch_in = x @ W_in.T + b_in                  # (B,T,H*BR)
  per t:  i_d = beta*i_d + (1-beta)*branch_in_t  # beta = sigmoid(tau_n), (H,BR)
          v   = alpha*v + (1-alpha)*i_d.sum(br)  # alpha = sigmoid(tau_m), (H,)
          spike = (v >= 1); v -= spike; counts += spike
  out = counts @ W_out.T + b_out                 # (B,D_OUT)

Strategy: data-parallel over batch across 8 cores (32 rows each). Per core,
T=500 is processed in chunks pipelined across engines.

GEMM: fp16 "hh" pass (W_hi @ x_hi, 6 k-tiles) into PSUM P0, plus fp8e4m3
DoubleRow correction products into PSUM P1. The corrections recover the
fp16-split residuals (W_lo @ x_hi + W_hi @ x_lo) from power-of-2-scaled fp8
operands; DoubleRow runs fp8 matmuls at 0.5 cycles/row with 256-deep
contraction, so each correction product costs 1/4 of an fp16 pass. The W
operands are UNSCALED (uniform row magnitudes) -- folding the per-row
(1-beta)(1-alpha) scale into W pushes the fp8 splits into subnormal flush and
fails the accuracy gate; instead sc2 stays in the Act epilogue:
  Act#1: u0 = sc2*P0 + b2   (baseline epilogue)
  Act#2: c  = (sc2*2^-17)*P1   (per-partition scale AP)
  Pool:  u  = u0 + c
Host emulation: 2-product corrections reproduce the fp32 reference's spike
train exactly on this data (rel 1e-7, same as the fp16 3-pass baseline).

Engines:
  PE  : hh fp16 matmuls + fp8 DR corrections + readout
  Act : PSUM epilogues (two per (m,g)), Sign() spike-compare on hist
  Pool: epilogue combine adds, branch-sum adds
  DVE : IIR scans, carry handling, serial per-timestep spike loop,
        spike-count time reduction
"""

import sys

if "/opt/trn_rl_repo" not in sys.path:
    sys.path.insert(0, "/opt/trn_rl_repo")

from contextlib import ExitStack

import numpy as np
import ml_dtypes

import concourse.bass as bass
import concourse.mybir as mybir
import concourse.tile as tile
from concourse import bacc

B, T, D_IN, H, BR, D_OUT = 256, 500, 700, 200, 2, 35
NCORES = 8
BL = B // NCORES          # local batch = 32
NK = 6                    # k-tiles; D_IN padded 700 -> 768 so every tile is 128
DP = NK * 128             # padded contraction dim (768)
M = 4                     # m-tiles, m=(br,j): o'' = m*128 + p, h = (m%2)*128+p
OP = M * 128              # padded output rows (512)
NJ = 2                    # h groups (j=0: h<128, j=1: h 128..199)
NF = NJ * BL              # spike-loop state columns (64)
BG = 8                    # batches per matmul n-group
NG = BL // BG             # 4 n-groups

CHUNKS = (50,) * 9 + (25, 15, 10)     # sum = T; shrinking tail chunks

# fp8 correction scheme: NPROD products (wvar, xvar); all products land at
# scale PSCALE = 2^16 in the SAME PSUM bank as the fp16 hh pass, whose weights
# are pre-scaled by 2^16 (exact power-of-2 in fp16; max |W|*2^16 ~ 13K < 65504).
# The single Act epilogue applies sc2*2^-16.
SXA, SWH = 2.0 ** 11, 2.0 ** 5        # x_lo*SXA, W_hi*SWH
SXH, SWL = 2.0 ** 4, 2.0 ** 12        # x_hi*SXH, W_lo*SWL
PSCALE = 2.0 ** 16
E4M3 = ml_dtypes.float8_e4m3

# product tables by nprod: (w split index, x split index)
# w splits: 0=wh1, 1=wh2, 2=wa, 3=wb ; x splits: 0=xa, 1=xb, 2=xh1, 3=xh2
PRODUCTS = {
    2: [(0, 0), (2, 2)],
    6: [(0, 0), (0, 1), (1, 0), (2, 2), (3, 2), (2, 3)],
}
XVARS = {2: [0, 2], 6: [0, 1, 2, 3]}  # distinct x splits shipped


def _f32(a):
    return np.ascontiguousarray(a, dtype=np.float32)


def _build(T_, chunks, nprod, alpha_uniform_val=None):
    chunks = tuple(chunks)
    NCH = len(chunks)
    assert sum(chunks) == T_
    C0 = max(chunks)
    sizes = sorted(set(chunks))
    prods = PRODUCTS[nprod]
    xvars = XVARS[nprod]
    NXV = len(xvars)
    xv_pos = {v: i for i, v in enumerate(xvars)}   # x split -> slot in x8
    wlist = sorted({p[0] for p in prods})          # distinct w splits shipped
    w_pos = {v: i for i, v in enumerate(wlist)}
    NW = len(wlist)

    # x columns per (chunk, group) and offsets in the flat tensors
    xoff16 = np.cumsum([0] + [NK * BG * cc for cc in chunks]).tolist()
    FT16 = xoff16[-1]
    xoff8 = np.cumsum([0] + [NK * NXV * BG * cc for cc in chunks]).tolist()
    FT8 = xoff8[-1]
    # d0 blocks per distinct chunk size
    soff = {}
    off = 0
    for s in sizes:
        soff[s] = off
        off += M * BL * s
    SD = off

    fp32 = mybir.dt.float32
    fp16 = mybir.dt.float16
    fp8 = mybir.dt.float8e4
    AF = mybir.ActivationFunctionType
    AL = mybir.AluOpType
    PM = mybir.MatmulPerfMode

    nc = bacc.Bacc("TRN2", target_bir_lowering=False, debug=False,
                   num_devices=NCORES)

    xt_d = nc.dram_tensor("xt", [NG, 128, FT16], fp16, kind="ExternalInput")
    x8_d = nc.dram_tensor("x8", [NG, 128, FT8], fp8, kind="ExternalInput")
    wt_d = nc.dram_tensor("wt", [NK, 128, OP], fp16, kind="ExternalInput")
    w8_d = nc.dram_tensor("w8", [NW, NK, 128, OP], fp8, kind="ExternalInput")
    sc2_d = nc.dram_tensor("sc2", [128, M], fp32, kind="ExternalInput")
    sc28_d = nc.dram_tensor("sc28", [128, M], fp32, kind="ExternalInput")
    b2_d = nc.dram_tensor("b2", [128, M], fp32, kind="ExternalInput")
    bt_d = nc.dram_tensor("bt", [128, M], fp32, kind="ExternalInput")
    atile_d = nc.dram_tensor("atile", [128, NF], fp32, kind="ExternalInput")
    woutT_d = nc.dram_tensor("woutT", [2 * 128, D_OUT], fp32, kind="ExternalInput")
    bout_d = nc.dram_tensor("bout", [D_OUT, 1], fp32, kind="ExternalInput")

    out_d = nc.dram_tensor("out", [D_OUT, BL], fp32, kind="ExternalOutput")
    tok_d = nc.dram_tensor("tok", [1, 16], fp32, kind="ExternalInput")
    tok_o = nc.dram_tensor("tok_out", [1, 16], fp32, kind="ExternalOutput")

    with tile.TileContext(nc) as tc, ExitStack() as ctx:
        const = ctx.enter_context(tc.tile_pool(name="const", bufs=1))
        st = ctx.enter_context(tc.tile_pool(name="state", bufs=1))
        up = ctx.enter_context(tc.tile_pool(name="up", bufs=2))
        wp = ctx.enter_context(tc.tile_pool(name="wph", bufs=2))
        hp = ctx.enter_context(tc.tile_pool(name="hp", bufs=2))
        xp = ctx.enter_context(tc.tile_pool(name="xin", bufs=3))
        ps = ctx.enter_context(tc.tile_pool(name="psum", bufs=2, space="PSUM"))
        pso = ctx.enter_context(tc.tile_pool(name="psout", bufs=1, space="PSUM"))
        scr = ctx.enter_context(tc.tile_pool(name="scr", bufs=2))

        w_sb = const.tile([128, NK * OP], fp16, tag="wsb")
        nc.sync.dma_start(
            w_sb[:].rearrange("p (k o) -> p k o", k=NK),
            wt_d.ap().rearrange("k p o -> p k o"))
        w8_sb = const.tile([128, NW * NK * OP], fp8, tag="w8sb")
        nc.scalar.dma_start(
            w8_sb[:].rearrange("p (q k o) -> p q k o", q=NW, k=NK),
            w8_d.ap().rearrange("q k p o -> p q k o"))
        sc2 = const.tile([128, M], fp32)
        nc.sync.dma_start(sc2[:], sc2_d.ap())
        sc28 = const.tile([128, M], fp32)
        nc.sync.dma_start(sc28[:], sc28_d.ap())
        b2 = const.tile([128, M], fp32)
        nc.sync.dma_start(b2[:], b2_d.ap())
        bt = const.tile([128, M], fp32)
        nc.scalar.dma_start(bt[:], bt_d.ap())
        atile = const.tile([128, NF], fp32)
        nc.scalar.dma_start(atile[:], atile_d.ap())
        # d0 (scan multipliers: beta everywhere, 0 at each batch's t=0)
        d0_sb = const.tile([128, SD], fp32)
        nc.vector.memset(d0_sb[:], 0.0)
        for si, s in enumerate(sizes):
            for m in range(M):
                blk = d0_sb[:, soff[s] + m * BL * s:
                            soff[s] + (m + 1) * BL * s]
                nc.scalar.activation(blk, blk, AF.Identity,
                                     bias=bt[:, m:m + 1], scale=0.0)
                nc.vector.memset(
                    blk.rearrange("p (b c) -> p b c", c=s)[:, :, 0], 0.0)
        woutT_sb = const.tile([128, 2 * D_OUT], fp32)
        nc.scalar.dma_start(woutT_sb[:, 0:D_OUT], woutT_d.ap()[0:128])
        nc.scalar.dma_start(woutT_sb[:, D_OUT:2 * D_OUT], woutT_d.ap()[128:256])
        bout_sb = const.tile([D_OUT, 1], fp32)
        nc.scalar.dma_start(bout_sb[:], bout_d.ap())
        negone = const.tile([128, 1], fp32, tag="negone")
        nc.vector.memset(negone[:], -1.0)

        w8v = w8_sb[:].rearrange("p (q k o) -> p q k o", q=NW, k=NK)

        def cmp_phase(hist, cc):
            hv = hist[:, 0:cc * NF]
            nc.scalar.activation(hv, hv, AF.Sign,
                                 bias=negone[:, 0:1], scale=negone[:, 0:1])

        def red_phase(hist, cc, counts):
            csc = scr.tile([128, NF], fp32, tag="csc")
            nc.vector.tensor_reduce(
                csc[:], hist[:, 0:cc * NF].rearrange("p (c f) -> p f c", f=NF),
                mybir.AxisListType.X, AL.add)
            nc.vector.tensor_tensor(counts[:], counts[:], csc[:], AL.add)

        MORDER = (0, 2, 1, 3)   # m emission order; pairs (0,2),(1,3) feed j0,j1
        SP = 256                # scan piece length (columns)

        def emit_gemm(c, CC, u):
            """GEMM for chunk c into tile u. n-groups processed in pairs
            sharing each weight load; m-tiles in MORDER so the branch-sum
            inputs complete early-first."""
            NNc = BG * CC
            ndr = len(prods) * (NK // 2)
            for gp in range(NG // 2):
                xs16 = []
                xs8 = []
                for gi in range(2):
                    g = 2 * gp + gi
                    x16 = xp.tile([128, NK * BG * C0], fp16, tag="x16")
                    nc.sync.dma_start(
                        x16[:, 0:NK * NNc],
                        xt_d.ap()[g][:, xoff16[c]:xoff16[c + 1]])
                    x8 = xp.tile([128, NK * NXV * BG * C0], fp8, tag="x8")
                    nc.sync.dma_start(
                        x8[:, 0:NK * NXV * NNc],
                        x8_d.ap()[g][:, xoff8[c]:xoff8[c + 1]])
                    xs16.append(x16)
                    xs8.append(x8[:, 0:NK * NXV * NNc].rearrange(
                        "p (k v n) -> p k v n", k=NK, v=NXV))
                for m in MORDER:
                    pts = [ps.tile([128, NNc], fp32, tag=f"pt{gi}",
                                   name=f"pt{gi}") for gi in range(2)]
                    for k in range(NK):
                        wap = w_sb[:, k * OP + m * 128:
                                   k * OP + (m + 1) * 128]
                        for gi in range(2):
                            nc.tensor.matmul(
                                pts[gi][:], wap,
                                xs16[gi][:, k * NNc:(k + 1) * NNc],
                                start=(k == 0), stop=False)
                    i = 0
                    for (wv, xv) in prods:
                        for j in range(NK // 2):
                            w8ap = w8v[:, w_pos[wv], 2 * j:2 * j + 2,
                                       m * 128:(m + 1) * 128]
                            for gi in range(2):
                                nc.tensor.matmul(
                                    pts[gi][:], w8ap,
                                    xs8[gi][:, 2 * j:2 * j + 2, xv_pos[xv]],
                                    start=False, stop=(i == ndr - 1),
                                    perf_mode=PM.DoubleRow)
                            i += 1
                    for gi in range(2):
                        g = 2 * gp + gi
                        nc.scalar.activation(
                            u[:, m * BL * C0 + g * NNc:
                              m * BL * C0 + (g + 1) * NNc],
                            pts[gi][:], AF.Identity,
                            bias=b2[:, m:m + 1], scale=sc28[:, m:m + 1])

        def scan_ops(c, CC, u, carry, sp=SP):
            """DVE ops for chunk c's dendrite IIR, chopped into pieces that
            interleave into the previous chunk's spike chain."""
            ops = []
            L = BL * CC
            for m in MORDER:
                base = m * BL * C0
                um = u[:, base:base + L]
                um3 = um.rearrange("p (b c) -> p b c", c=CC)
                d0c = soff[CC] + m * BL * CC
                if c > 0:
                    def carry_add(m=m, um3=um3):
                        nc.vector.scalar_tensor_tensor(
                            um3[:, :, 0], carry[:, m * BL:(m + 1) * BL],
                            bt[:, m:m + 1], um3[:, :, 0], AL.mult, AL.add)
                    ops.append(carry_add)
                p0 = 0
                while p0 < L:
                    p1 = min(p0 + sp, L)
                    def piece(p0=p0, p1=p1, um=um, d0c=d0c):
                        init = 0.0 if p0 == 0 else um[:, p0 - 1:p0]
                        nc.vector.tensor_tensor_scan(
                            um[:, p0:p1], d0_sb[:, d0c + p0:d0c + p1],
                            um[:, p0:p1], init, AL.mult, AL.add)
                    ops.append(piece)
                    p0 = p1
                if c < NCH - 1:
                    def carry_copy(m=m, um3=um3, CC=CC):
                        nc.vector.tensor_copy(carry[:, m * BL:(m + 1) * BL],
                                              um3[:, :, CC - 1])
                    ops.append(carry_copy)
            return ops

        def emit_adds(CC, u, wti, halves):
            """Branch sums w_j = i'_d[j] + i'_d[2+j] on Pool, optionally in
            batch-halves so the early halves start before all scans finish."""
            wre = wti[:, 0:CC * NF].rearrange("p (c j b) -> p b j c",
                                              j=NJ, b=BL)
            spans = [(0, BL // 2), (BL // 2, BL)] if halves else [(0, BL)]
            for j in range(NJ):
                ua = u[:, j * BL * C0:j * BL * C0 + BL * CC].rearrange(
                    "p (b c) -> p b c", c=CC)
                ub = u[:, (2 + j) * BL * C0:(2 + j) * BL * C0 + BL * CC
                       ].rearrange("p (b c) -> p b c", c=CC)
                for (b0, b1) in spans:
                    nc.gpsimd.tensor_tensor(
                        wre[:, b0:b1, j, :], ua[:, b0:b1], ub[:, b0:b1],
                        AL.add)

        def body_once():
            vst = st.tile([128, NF], fp32, tag="vst")  # negated potential
            counts = st.tile([128, NF], fp32, tag="cnt")
            carry = st.tile([128, M * BL], fp32, tag="carry")
            nc.vector.memset(vst[:], 0.0)
            nc.vector.memset(counts[:], 0.0)

            # prologue: chunk 0's GEMM + scans + branch sums up front
            u_cur = up.tile([128, M * BL * C0], fp32, tag="u", name="u0")
            emit_gemm(0, chunks[0], u_cur)
            for op in scan_ops(0, chunks[0], u_cur, carry, sp=BL * chunks[0]):
                op()
            wti_cur = wp.tile([128, C0 * NF], fp32, tag="wti", name="wti0")
            emit_adds(chunks[0], u_cur, wti_cur, halves=False)

            prev = None     # (hist, chunk_len) of previous chunk
            for c, CC in enumerate(chunks):
                # previous chunk's spike-compare + count-reduce first so they
                # sit ahead of the next GEMM's epilogues in the Act/DVE queues
                if prev is not None:
                    cmp_phase(*prev)
                    red_phase(prev[0], prev[1], counts)

                # next chunk's GEMM + its scan pieces (interleaved below)
                if c + 1 < NCH:
                    CCn = chunks[c + 1]
                    u_next = up.tile([128, M * BL * C0], fp32, tag="u",
                                     name="un")
                    emit_gemm(c + 1, CCn, u_next)
                    pieces = scan_ops(c + 1, CCn, u_next, carry)
                else:
                    u_next = None
                    pieces = []

                hist = hp.tile([128, C0 * NF], fp32, tag="hist")

                # -- spike loop (negated state: vt = -v), scan pieces of the
                # next chunk drip-fed into the semaphore gaps --
                slots = 2 * CC
                start = slots // 2
                n_ops = len(pieces)
                idx = 0
                slot = 0

                def drain():
                    nonlocal idx
                    while (idx < n_ops and
                           idx * (slots - start) <=
                           (slot - start) * n_ops):
                        pieces[idx]()
                        idx += 1

                for t in range(CC):
                    tA = hist[:, t * NF:(t + 1) * NF]   # pre-reset vt' kept
                    wt_t = wti_cur[:, t * NF:(t + 1) * NF]
                    if alpha_uniform_val is not None:
                        nc.vector.scalar_tensor_tensor(
                            tA, vst[:], float(alpha_uniform_val),
                            wt_t, AL.mult, AL.subtract)
                    else:
                        nc.vector.tensor_tensor(tA, vst[:], atile[:], AL.mult)
                        nc.vector.tensor_tensor(tA, tA, wt_t, AL.subtract)
                    slot += 1
                    if slot > start:
                        drain()
                    nc.vector.scalar_tensor_tensor(
                        vst[:], tA, -1.0, tA, AL.is_le, AL.add)
                    slot += 1
                    if slot > start:
                        drain()
                while idx < n_ops:
                    pieces[idx]()
                    idx += 1

                if c + 1 < NCH:
                    wti_next = wp.tile([128, C0 * NF], fp32, tag="wti",
                                       name="wtin")
                    emit_adds(chunks[c + 1], u_next, wti_next, halves=True)
                    u_cur, wti_cur = u_next, wti_next
                prev = (hist, CC)

            # final chunk: compare inline on DVE
            fh, fcc = prev
            nc.vector.tensor_scalar(fh[:, 0:fcc * NF], fh[:, 0:fcc * NF],
                                    -1.0, None, AL.is_le)
            csc = scr.tile([128, NF], fp32, tag="csc")
            nc.vector.tensor_reduce(
                csc[:], fh[:, 0:fcc * NF].rearrange("p (c f) -> p f c", f=NF),
                mybir.AxisListType.X, AL.add)
            nc.vector.scalar_tensor_tensor(
                counts[:], csc[:], 2.0, counts[:], AL.mult, AL.add)

            # -- readout --
            po = pso.tile([D_OUT, BL], fp32, tag="po")
            nc.tensor.matmul(po[:], woutT_sb[:, 0:D_OUT], counts[:, 0:BL],
                             start=True, stop=False)
            nc.tensor.matmul(po[:], woutT_sb[0:H - 128, D_OUT:2 * D_OUT],
                             counts[0:H - 128, BL:2 * BL], start=False,
                             stop=True)
            out_sb = scr.tile([D_OUT, BL], fp32, tag="osb")
            nc.scalar.activation(out_sb[:], po[:], AF.Identity,
                                 bias=bout_sb[:, 0:1], scale=1.0)
            nc.sync.dma_start(out_d.ap(), out_sb[:])

        body_once()
        tok_sb = scr.tile([1, 16], fp32, tag="tok")
        nc.sync.dma_start(tok_sb[:], tok_d.ap())
        nc.sync.dma_start(tok_o.ap(), tok_sb[:])

    nc.compile()
    return nc


def _prep_host(x, W_in, b_in, tau_n, tau_m, W_out, b_out, T_, chunks, nprod):
    """Host-side constant prep. Returns (shared_inputs, per_core_inputs, alpha_uni)."""
    x = _f32(x); W_in = _f32(W_in); b_in = _f32(b_in)
    tau_n = _f32(tau_n); tau_m = _f32(tau_m)
    W_out = _f32(W_out); b_out = _f32(b_out)
    chunks = tuple(chunks)
    assert sum(chunks) == T_
    prods = PRODUCTS[nprod]
    xvars = XVARS[nprod]
    NXV = len(xvars)
    NW = len({p[0] for p in prods})

    beta = _f32(1.0 / (1.0 + np.exp(-tau_n.astype(np.float64))))   # (H,BR)
    alpha = _f32(1.0 / (1.0 + np.exp(-tau_m.astype(np.float64))))  # (H,)
    one = np.float32(1.0)

    def fp8q(a):
        return np.ascontiguousarray(a, dtype=np.float32).astype(E4M3)

    # m-tile map: m=(br,j) -> rows p: h = (m%2)*128+p, o = h*BR + br
    wt = np.zeros((NK, 128, OP), np.float32)
    sc2 = np.zeros((128, M), np.float32)
    sc28 = np.zeros((128, M), np.float32)
    b2 = np.zeros((128, M), np.float32)
    bt = np.zeros((128, M), np.float32)
    for m in range(M):
        br, j = m // 2, m % 2
        for p in range(128):
            h = j * 128 + p
            if h >= H:
                continue
            o = h * BR + br
            s = (one - beta[h, br]) * (one - alpha[h])
            sc2[p, m] = s
            sc28[p, m] = s * np.float32(1.0 / PSCALE)
            b2[p, m] = s * b_in[o]
            bt[p, m] = beta[h, br]
            wrow = np.zeros(DP, np.float32)
            wrow[:D_IN] = W_in[o]
            wt[:, :, m * 128 + p] = wrow.reshape(NK, 128)
    wh = wt.astype(np.float16).astype(np.float32)
    wl = wt - wh
    # w splits: 0=wh1, 1=wh2, 2=wa, 3=wb (scaled, stored fp8)
    wh1 = fp8q(wh * np.float32(SWH))
    wh2 = fp8q(wh * np.float32(SWH) - wh1.astype(np.float32))
    wa = fp8q(wl * np.float32(SWL))
    wb = fp8q(wl * np.float32(SWL) - wa.astype(np.float32))
    wsplit_all = [wh1, wh2, wa, wb]
    w8 = np.stack([wsplit_all[i] for i in sorted({p[0] for p in prods})])

    atile = np.zeros((128, NF), np.float32)
    for j in range(NJ):
        for p in range(128):
            h = j * 128 + p
            if h >= H:
                continue
            atile[p, j * BL:(j + 1) * BL] = alpha[h]
    woutT = np.zeros((256, D_OUT), np.float32)
    woutT[:H, :] = 0.5 * W_out.T
    teff = np.float32(T_ - chunks[-1])
    bout = (b_out + 0.5 * teff * W_out.sum(axis=1)).reshape(D_OUT, 1)

    shared = dict(wt=(wt * np.float32(PSCALE)).astype(np.float16),
                  w8=w8.view(np.uint8),
                  sc2=sc2, sc28=sc28, b2=b2, bt=bt, atile=atile,
                  woutT=_f32(woutT), bout=_f32(bout))

    percore = []
    for core in range(NCORES):
        xl_ = x[core * BL:(core + 1) * BL, :T_, :]        # (BL,T,D_IN)
        xp_ = np.zeros((BL, T_, DP), np.float32)
        xp_[:, :, :D_IN] = xl_
        xh = xp_.astype(np.float16).astype(np.float32)
        xlo = xp_ - xh
        # x splits: 0=xa, 1=xb, 2=xh1, 3=xh2
        xa = fp8q(xlo * np.float32(SXA))
        xsplit = {0: xa}
        if 1 in xvars:
            xsplit[1] = fp8q(xlo * np.float32(SXA) - xa.astype(np.float32))
        xh1 = fp8q(xh * np.float32(SXH))
        xsplit[2] = xh1
        if 3 in xvars:
            xsplit[3] = fp8q(xh * np.float32(SXH) - xh1.astype(np.float32))

        FT16 = sum(NK * BG * cc for cc in chunks)
        FT8 = sum(NK * NXV * BG * cc for cc in chunks)
        xt = np.zeros((NG, 128, FT16), np.float16)
        x8 = np.zeros((NG, 128, FT8), E4M3)
        colo16 = 0
        colo8 = 0
        t0 = 0
        xh16 = xp_.astype(np.float16)
        for cc in chunks:
            for g in range(NG):
                sub = xh16[g * BG:(g + 1) * BG, t0:t0 + cc, :]   # (BG,cc,DP)
                sg = sub.reshape(BG, cc, NK, 128).transpose(3, 2, 0, 1)
                xt[g, :, colo16:colo16 + NK * BG * cc] = sg.reshape(128, -1)
                # x8 layout: (k, v, b, t)
                blk = np.empty((128, NK, NXV, BG, cc), E4M3)
                for vi, v in enumerate(xvars):
                    sv = xsplit[v][g * BG:(g + 1) * BG, t0:t0 + cc, :]
                    blk[:, :, vi] = sv.reshape(BG, cc, NK, 128).transpose(
                        3, 2, 0, 1)
                x8[g, :, colo8:colo8 + NK * NXV * BG * cc] = blk.reshape(128, -1)
            colo16 += NK * BG * cc
            colo8 += NK * NXV * BG * cc
            t0 += cc
        percore.append(dict(xt=xt, x8=x8.view(np.uint8)))
    uni = float(alpha[0]) if np.all(alpha == alpha[0]) else None
    return shared, percore, uni


TRACE = False          # set by test harness for profiling runs
LAST_RESULT = None
NPROD = 2


def kernel(x, W_in, b_in, tau_n, tau_m, W_out, b_out):
    global LAST_RESULT
    from concourse.bass_utils import run_bass_kernel_spmd

    shared, percore, uni = _prep_host(x, W_in, b_in, tau_n, tau_m, W_out,
                                      b_out, T, CHUNKS, NPROD)
    nc = _build(T, CHUNKS, NPROD, alpha_uniform_val=uni)
    tok = np.zeros((1, 16), np.float32)
    in_maps = [dict(shared, tok=tok, **percore[core])
               for core in range(NCORES)]
    res = run_bass_kernel_spmd(nc, in_maps, core_ids=list(range(NCORES)),
                               trace=TRACE)
    LAST_RESULT = res
    out = np.empty((B, D_OUT), np.float32)
    for core in range(NCORES):
        out[core * BL:(core + 1) * BL, :] = res.results[core]["out"].T
    return out


# revision 17
# speedup vs baseline: 1.0570x; 1.0335x over previous
"""Trainium2 Bass kernel for the dendritic-branch spiking FNN (DH_SFNN).

Model (per reference):
  branch_in = x @ W_in.T + b_in                  # (B,T,H*BR)
  per t:  i_d = beta*i_d + (1-beta)*branch_in_t  # beta = sigmoid(tau_n), (H,BR)
          v   = alpha*v + (1-alpha)*i_d.sum(br)  # alpha = sigmoid(tau_m), (H,)
          spike = (v >= 1); v -= spike; counts += spike
  out = counts @ W_out.T + b_out                 # (B,D_OUT)

Strategy: data-parallel over batch across 8 cores (32 rows each). Per core,
T=500 is processed in chunks pipelined across engines.

GEMM: fp16 "hh" pass (W_hi @ x_hi, 6 k-tiles) into PSUM P0, plus fp8e4m3
DoubleRow correction products into PSUM P1. The corrections recover the
fp16-split residuals (W_lo @ x_hi + W_hi @ x_lo) from power-of-2-scaled fp8
operands; DoubleRow runs fp8 matmuls at 0.5 cycles/row with 256-deep
contraction, so each correction product costs 1/4 of an fp16 pass. The W
operands are UNSCALED (uniform row magnitudes) -- folding the per-row
(1-beta)(1-alpha) scale into W pushes the fp8 splits into subnormal flush and
fails the accuracy gate; instead sc2 stays in the Act epilogue:
  Act#1: u0 = sc2*P0 + b2   (baseline epilogue)
  Act#2: c  = (sc2*2^-17)*P1   (per-partition scale AP)
  Pool:  u  = u0 + c
Host emulation: 2-product corrections reproduce the fp32 reference's spike
train exactly on this data (rel 1e-7, same as the fp16 3-pass baseline).

Engines:
  PE  : hh fp16 matmuls + fp8 DR corrections + readout
  Act : PSUM epilogues (two per (m,g)), Sign() spike-compare on hist
  Pool: epilogue combine adds, branch-sum adds
  DVE : IIR scans, carry handling, serial per-timestep spike loop,
        spike-count time reduction
"""

import sys

if "/opt/trn_rl_repo" not in sys.path:
    sys.path.insert(0, "/opt/trn_rl_repo")

from contextlib import ExitStack

import numpy as np
import ml_dtypes

import concourse.bass as bass
import concourse.mybir as mybir
import concourse.tile as tile
from concourse import bacc

B, T, D_IN, H, BR, D_OUT = 256, 500, 700, 200, 2, 35
NCORES = 8
BL = B // NCORES          # local batch = 32
NK = 6                    # k-tiles; D_IN padded 700 -> 768 so every tile is 128
DP = NK * 128             # padded contraction dim (768)
M = 4                     # m-tiles, m=(br,j): o'' = m*128 + p, h = (m%2)*128+p
OP = M * 128              # padded output rows (512)
NJ = 2                    # h groups (j=0: h<128, j=1: h 128..199)
NF = NJ * BL              # spike-loop state columns (64)
BG = 8                    # batches per matmul n-group
NG = BL // BG             # 4 n-groups

CHUNKS = (50,) * 9 + (25, 15, 10)     # sum = T; shrinking tail chunks

# fp8 correction scheme: NPROD products (wvar, xvar); all products land at
# scale PSCALE = 2^16 in the SAME PSUM bank as the fp16 hh pass, whose weights
# are pre-scaled by 2^16 (exact power-of-2 in fp16; max |W|*2^16 ~ 13K < 65504).
# The single Act epilogue applies sc2*2^-16.
SXA, SWH = 2.0 ** 11, 2.0 ** 5        # x_lo*SXA, W_hi*SWH
SXH, SWL = 2.0 ** 4, 2.0 ** 12        # x_hi*SXH, W_lo*SWL
PSCALE = 2.0 ** 16
E4M3 = ml_dtypes.float8_e4m3

# product tables by nprod: (w split index, x split index)
# w splits: 0=wh1, 1=wh2, 2=wa, 3=wb ; x splits: 0=xa, 1=xb, 2=xh1, 3=xh2
PRODUCTS = {
    2: [(0, 0), (2, 2)],
    6: [(0, 0), (0, 1), (1, 0), (2, 2), (3, 2), (2, 3)],
}
XVARS = {2: [0, 2], 6: [0, 1, 2, 3]}  # distinct x splits shipped


def _f32(a):
    return np.ascontiguousarray(a, dtype=np.float32)


def _build(T_, chunks, nprod, alpha_uniform_val=None):
    chunks = tuple(chunks)
    NCH = len(chunks)
    assert sum(chunks) == T_
    C0 = max(chunks)
    sizes = sorted(set(chunks))
    prods = PRODUCTS[nprod]
    xvars = XVARS[nprod]
    NXV = len(xvars)
    xv_pos = {v: i for i, v in enumerate(xvars)}   # x split -> slot in x8
    wlist = sorted({p[0] for p in prods})          # distinct w splits shipped
    w_pos = {v: i for i, v in enumerate(wlist)}
    NW = len(wlist)

    # x columns per (chunk, group) and offsets in the flat tensors
    xoff16 = np.cumsum([0] + [NK * BG * cc for cc in chunks]).tolist()
    FT16 = xoff16[-1]
    xoff8 = np.cumsum([0] + [NK * NXV * BG * cc for cc in chunks]).tolist()
    FT8 = xoff8[-1]
    # d0 blocks per distinct chunk size
    soff = {}
    off = 0
    for s in sizes:
        soff[s] = off
        off += M * BL * s
    SD = off

    fp32 = mybir.dt.float32
    fp16 = mybir.dt.float16
    fp8 = mybir.dt.float8e4
    AF = mybir.ActivationFunctionType
    AL = mybir.AluOpType
    PM = mybir.MatmulPerfMode

    nc = bacc.Bacc("TRN2", target_bir_lowering=False, debug=False,
                   num_devices=NCORES)

    xt_d = nc.dram_tensor("xt", [NG, 128, FT16], fp16, kind="ExternalInput")
    x8_d = nc.dram_tensor("x8", [NG, 128, FT8], fp8, kind="ExternalInput")
    wt_d = nc.dram_tensor("wt", [NK, 128, OP], fp16, kind="ExternalInput")
    w8_d = nc.dram_tensor("w8", [NW, NK, 128, OP], fp8, kind="ExternalInput")
    sc2_d = nc.dram_tensor("sc2", [128, M], fp32, kind="ExternalInput")
    sc28_d = nc.dram_tensor("sc28", [128, M], fp32, kind="ExternalInput")
    b2_d = nc.dram_tensor("b2", [128, M], fp32, kind="ExternalInput")
    bt_d = nc.dram_tensor("bt", [128, M], fp32, kind="ExternalInput")
    atile_d = nc.dram_tensor("atile", [128, NF], fp32, kind="ExternalInput")
    woutT_d = nc.dram_tensor("woutT", [2 * 128, D_OUT], fp32, kind="ExternalInput")
    bout_d = nc.dram_tensor("bout", [D_OUT, 1], fp32, kind="ExternalInput")

    out_d = nc.dram_tensor("out", [D_OUT, BL], fp32, kind="ExternalOutput")
    tok_d = nc.dram_tensor("tok", [1, 16], fp32, kind="ExternalInput")
    tok_o = nc.dram_tensor("tok_out", [1, 16], fp32, kind="ExternalOutput")

    with tile.TileContext(nc) as tc, ExitStack() as ctx:
        const = ctx.enter_context(tc.tile_pool(name="const", bufs=1))
        st = ctx.enter_context(tc.tile_pool(name="state", bufs=1))
        up = ctx.enter_context(tc.tile_pool(name="up", bufs=2))
        wp = ctx.enter_context(tc.tile_pool(name="wph", bufs=2))
        hp = ctx.enter_context(tc.tile_pool(name="hp", bufs=2))
        xp = ctx.enter_context(tc.tile_pool(name="xin", bufs=3))
        ps = ctx.enter_context(tc.tile_pool(name="psum", bufs=2, space="PSUM"))
        pso = ctx.enter_context(tc.tile_pool(name="psout", bufs=1, space="PSUM"))
        scr = ctx.enter_context(tc.tile_pool(name="scr", bufs=2))

        w_sb = const.tile([128, NK * OP], fp16, tag="wsb")
        nc.sync.dma_start(
            w_sb[:].rearrange("p (k o) -> p k o", k=NK),
            wt_d.ap().rearrange("k p o -> p k o"))
        w8_sb = const.tile([128, NW * NK * OP], fp8, tag="w8sb")
        nc.scalar.dma_start(
            w8_sb[:].rearrange("p (q k o) -> p q k o", q=NW, k=NK),
            w8_d.ap().rearrange("q k p o -> p q k o"))
        sc2 = const.tile([128, M], fp32)
        nc.sync.dma_start(sc2[:], sc2_d.ap())
        sc28 = const.tile([128, M], fp32)
        nc.sync.dma_start(sc28[:], sc28_d.ap())
        b2 = const.tile([128, M], fp32)
        nc.sync.dma_start(b2[:], b2_d.ap())
        bt = const.tile([128, M], fp32)
        nc.scalar.dma_start(bt[:], bt_d.ap())
        atile = const.tile([128, NF], fp32)
        nc.scalar.dma_start(atile[:], atile_d.ap())
        # d0 (scan multipliers: beta everywhere, 0 at each batch's t=0)
        d0_sb = const.tile([128, SD], fp32)
        nc.vector.memset(d0_sb[:], 0.0)
        for si, s in enumerate(sizes):
            for m in range(M):
                blk = d0_sb[:, soff[s] + m * BL * s:
                            soff[s] + (m + 1) * BL * s]
                nc.scalar.activation(blk, blk, AF.Identity,
                                     bias=bt[:, m:m + 1], scale=0.0)
                nc.vector.memset(
                    blk.rearrange("p (b c) -> p b c", c=s)[:, :, 0], 0.0)
        woutT_sb = const.tile([128, 2 * D_OUT], fp32)
        nc.scalar.dma_start(woutT_sb[:, 0:D_OUT], woutT_d.ap()[0:128])
        nc.scalar.dma_start(woutT_sb[:, D_OUT:2 * D_OUT], woutT_d.ap()[128:256])
        bout_sb = const.tile([D_OUT, 1], fp32)
        nc.scalar.dma_start(bout_sb[:], bout_d.ap())
        negone = const.tile([128, 1], fp32, tag="negone")
        nc.vector.memset(negone[:], -1.0)

        w8v = w8_sb[:].rearrange("p (q k o) -> p q k o", q=NW, k=NK)

        def cmp_phase(hist, cc):
            hv = hist[:, 0:cc * NF]
            nc.scalar.activation(hv, hv, AF.Sign,
                                 bias=negone[:, 0:1], scale=negone[:, 0:1])

        def red_phase(hist, cc, counts):
            csc = scr.tile([128, NF], fp32, tag="csc")
            nc.vector.tensor_reduce(
                csc[:], hist[:, 0:cc * NF].rearrange("p (c f) -> p f c", f=NF),
                mybir.AxisListType.X, AL.add)
            nc.vector.tensor_tensor(counts[:], counts[:], csc[:], AL.add)

        MORDER = (0, 2, 1, 3)   # m emission order; pairs (0,2),(1,3) feed j0,j1
        SP = 256                # scan piece length (columns)

        def emit_gemm(c, CC, u):
            """GEMM for chunk c into tile u. n-groups processed in pairs
            sharing each weight load; m-tiles in MORDER so the branch-sum
            inputs complete early-first."""
            NNc = BG * CC
            ndr = len(prods) * (NK // 2)
            for gp in range(NG // 2):
                xs16 = []
                xs8 = []
                for gi in range(2):
                    g = 2 * gp + gi
                    x16 = xp.tile([128, NK * BG * C0], fp16, tag="x16")
                    nc.sync.dma_start(
                        x16[:, 0:NK * NNc],
                        xt_d.ap()[g][:, xoff16[c]:xoff16[c + 1]])
                    x8 = xp.tile([128, NK * NXV * BG * C0], fp8, tag="x8")
                    nc.sync.dma_start(
                        x8[:, 0:NK * NXV * NNc],
                        x8_d.ap()[g][:, xoff8[c]:xoff8[c + 1]])
                    xs16.append(x16)
                    xs8.append(x8[:, 0:NK * NXV * NNc].rearrange(
                        "p (k v n) -> p k v n", k=NK, v=NXV))
                for m in MORDER:
                    pts = [ps.tile([128, NNc], fp32, tag=f"pt{gi}",
                                   name=f"pt{gi}") for gi in range(2)]
                    for k in range(NK):
                        wap = w_sb[:, k * OP + m * 128:
                                   k * OP + (m + 1) * 128]
                        for gi in range(2):
                            nc.tensor.matmul(
                                pts[gi][:], wap,
                                xs16[gi][:, k * NNc:(k + 1) * NNc],
                                start=(k == 0), stop=False)
                    i = 0
                    for (wv, xv) in prods:
                        for j in range(NK // 2):
                            w8ap = w8v[:, w_pos[wv], 2 * j:2 * j + 2,
                                       m * 128:(m + 1) * 128]
                            for gi in range(2):
                                nc.tensor.matmul(
                                    pts[gi][:], w8ap,
                                    xs8[gi][:, 2 * j:2 * j + 2, xv_pos[xv]],
                                    start=False, stop=(i == ndr - 1),
                                    perf_mode=PM.DoubleRow)
                            i += 1
                    for gi in range(2):
                        g = 2 * gp + gi
                        nc.scalar.activation(
                            u[:, m * BL * C0 + g * NNc:
                              m * BL * C0 + (g + 1) * NNc],
                            pts[gi][:], AF.Identity,
                            bias=b2[:, m:m + 1], scale=sc28[:, m:m + 1])

        def scan_ops(c, CC, u, carry, sp=None):
            """DVE ops for chunk c's dendrite IIR, chopped into batch-aligned
            pieces. At every batch start d0 is 0 (or handled by the Pool
            carry-inject for c>0), so pieces take initial=0.0 and carry NO
            dependency on each other -- they interleave freely into the
            previous chunk's spike chain."""
            ops = []
            L = BL * CC
            if sp is None:
                sp = 4 * CC             # 4 batches per piece
            assert sp % CC == 0
            for m in MORDER:
                base = m * BL * C0
                um = u[:, base:base + L]
                um3 = um.rearrange("p (b c) -> p b c", c=CC)
                d0c = soff[CC] + m * BL * CC
                if c > 0:
                    # inject beta*carry into each batch's first column on
                    # Pool (carry was pre-scaled by beta at copy time)
                    def carry_add(m=m, um3=um3):
                        nc.gpsimd.tensor_tensor(
                            um3[:, :, 0], um3[:, :, 0],
                            carry[:, m * BL:(m + 1) * BL], AL.add)
                    ops.append(carry_add)
                p0 = 0
                while p0 < L:
                    p1 = min(p0 + sp, L)
                    def piece(p0=p0, p1=p1, um=um, d0c=d0c):
                        nc.vector.tensor_tensor_scan(
                            um[:, p0:p1], d0_sb[:, d0c + p0:d0c + p1],
                            um[:, p0:p1], 0.0, AL.mult, AL.add)
                    ops.append(piece)
                    p0 = p1
                if c < NCH - 1:
                    # carry := beta * i_d[last col]  (pre-scaled for inject)
                    def carry_copy(m=m, um3=um3, CC=CC):
                        nc.vector.tensor_scalar(
                            carry[:, m * BL:(m + 1) * BL], um3[:, :, CC - 1],
                            bt[:, m:m + 1], None, AL.mult)
                    ops.append(carry_copy)
            return ops

        def emit_adds(CC, u, wti, halves):
            """Branch sums w_j = i'_d[j] + i'_d[2+j] on Pool, optionally in
            batch-halves so the early halves start before all scans finish."""
            wre = wti[:, 0:CC * NF].rearrange("p (c j b) -> p b j c",
                                              j=NJ, b=BL)
            spans = [(0, BL // 2), (BL // 2, BL)] if halves else [(0, BL)]
            for j in range(NJ):
                ua = u[:, j * BL * C0:j * BL * C0 + BL * CC].rearrange(
                    "p (b c) -> p b c", c=CC)
                ub = u[:, (2 + j) * BL * C0:(2 + j) * BL * C0 + BL * CC
                       ].rearrange("p (b c) -> p b c", c=CC)
                for (b0, b1) in spans:
                    nc.gpsimd.tensor_tensor(
                        wre[:, b0:b1, j, :], ua[:, b0:b1], ub[:, b0:b1],
                        AL.add)

        def body_once():
            vst = st.tile([128, NF], fp32, tag="vst")  # negated potential
            counts = st.tile([128, NF], fp32, tag="cnt")
            carry = st.tile([128, M * BL], fp32, tag="carry")
            nc.vector.memset(vst[:], 0.0)
            nc.vector.memset(counts[:], 0.0)

            # prologue: chunk 0's GEMM + scans + branch sums up front
            u_cur = up.tile([128, M * BL * C0], fp32, tag="u", name="u0")
            emit_gemm(0, chunks[0], u_cur)
            for op in scan_ops(0, chunks[0], u_cur, carry, sp=BL * chunks[0]):
                op()
            wti_cur = wp.tile([128, C0 * NF], fp32, tag="wti", name="wti0")
            emit_adds(chunks[0], u_cur, wti_cur, halves=False)

            prev = None     # (hist, chunk_len) of previous chunk
            for c, CC in enumerate(chunks):
                # previous chunk's spike-compare + count-reduce first so they
                # sit ahead of the next GEMM's epilogues in the Act/DVE queues
                if prev is not None:
                    cmp_phase(*prev)
                    red_phase(prev[0], prev[1], counts)

                # next chunk's GEMM + its scan pieces (interleaved below)
                if c + 1 < NCH:
                    CCn = chunks[c + 1]
                    u_next = up.tile([128, M * BL * C0], fp32, tag="u",
                                     name="un")
                    emit_gemm(c + 1, CCn, u_next)
                    pieces = scan_ops(c + 1, CCn, u_next, carry)
                else:
                    u_next = None
                    pieces = []

                hist = hp.tile([128, C0 * NF], fp32, tag="hist")

                # -- spike loop (negated state: vt = -v), scan pieces of the
                # next chunk drip-fed into the semaphore gaps --
                slots = 2 * CC
                start = slots // 2
                n_ops = len(pieces)
                idx = 0
                slot = 0

                def drain():
                    nonlocal idx
                    while (idx < n_ops and
                           idx * (slots - start) <=
                           (slot - start) * n_ops):
                        pieces[idx]()
                        idx += 1

                for t in range(CC):
                    tA = hist[:, t * NF:(t + 1) * NF]   # pre-reset vt' kept
                    wt_t = wti_cur[:, t * NF:(t + 1) * NF]
                    if alpha_uniform_val is not None:
                        nc.vector.scalar_tensor_tensor(
                            tA, vst[:], float(alpha_uniform_val),
                            wt_t, AL.mult, AL.subtract)
                    else:
                        nc.vector.tensor_tensor(tA, vst[:], atile[:], AL.mult)
                        nc.vector.tensor_tensor(tA, tA, wt_t, AL.subtract)
                    slot += 1
                    if slot > start:
                        drain()
                    nc.vector.scalar_tensor_tensor(
                        vst[:], tA, -1.0, tA, AL.is_le, AL.add)
                    slot += 1
                    if slot > start:
                        drain()
                while idx < n_ops:
                    pieces[idx]()
                    idx += 1

                if c + 1 < NCH:
                    wti_next = wp.tile([128, C0 * NF], fp32, tag="wti",
                                       name="wtin")
                    emit_adds(chunks[c + 1], u_next, wti_next, halves=True)
                    u_cur, wti_cur = u_next, wti_next
                prev = (hist, CC)

            # final chunk: compare inline on DVE
            fh, fcc = prev
            nc.vector.tensor_scalar(fh[:, 0:fcc * NF], fh[:, 0:fcc * NF],
                                    -1.0, None, AL.is_le)
            csc = scr.tile([128, NF], fp32, tag="csc")
            nc.vector.tensor_reduce(
                csc[:], fh[:, 0:fcc * NF].rearrange("p (c f) -> p f c", f=NF),
                mybir.AxisListType.X, AL.add)
            nc.vector.scalar_tensor_tensor(
                counts[:], csc[:], 2.0, counts[:], AL.mult, AL.add)

            # -- readout --
            po = pso.tile([D_OUT, BL], fp32, tag="po")
            nc.tensor.matmul(po[:], woutT_sb[:, 0:D_OUT], counts[:, 0:BL],
                             start=True, stop=False)
            nc.tensor.matmul(po[:], woutT_sb[0:H - 128, D_OUT:2 * D_OUT],
                             counts[0:H - 128, BL:2 * BL], start=False,
                             stop=True)
            out_sb = scr.tile([D_OUT, BL], fp32, tag="osb")
            nc.scalar.activation(out_sb[:], po[:], AF.Identity,
                                 bias=bout_sb[:, 0:1], scale=1.0)
            nc.sync.dma_start(out_d.ap(), out_sb[:])

        body_once()
        tok_sb = scr.tile([1, 16], fp32, tag="tok")
        nc.sync.dma_start(tok_sb[:], tok_d.ap())
        nc.sync.dma_start(tok_o.ap(), tok_sb[:])

    nc.compile()
    return nc


def _prep_host(x, W_in, b_in, tau_n, tau_m, W_out, b_out, T_, chunks, nprod):
    """Host-side constant prep. Returns (shared_inputs, per_core_inputs, alpha_uni)."""
    x = _f32(x); W_in = _f32(W_in); b_in = _f32(b_in)
    tau_n = _f32(tau_n); tau_m = _f32(tau_m)
    W_out = _f32(W_out); b_out = _f32(b_out)
    chunks = tuple(chunks)
    assert sum(chunks) == T_
    prods = PRODUCTS[nprod]
    xvars = XVARS[nprod]
    NXV = len(xvars)
    NW = len({p[0] for p in prods})

    beta = _f32(1.0 / (1.0 + np.exp(-tau_n.astype(np.float64))))   # (H,BR)
    alpha = _f32(1.0 / (1.0 + np.exp(-tau_m.astype(np.float64))))  # (H,)
    one = np.float32(1.0)

    def fp8q(a):
        return np.ascontiguousarray(a, dtype=np.float32).astype(E4M3)

    # m-tile map: m=(br,j) -> rows p: h = (m%2)*128+p, o = h*BR + br
    wt = np.zeros((NK, 128, OP), np.float32)
    sc2 = np.zeros((128, M), np.float32)
    sc28 = np.zeros((128, M), np.float32)
    b2 = np.zeros((128, M), np.float32)
    bt = np.zeros((128, M), np.float32)
    for m in range(M):
        br, j = m // 2, m % 2
        for p in range(128):
            h = j * 128 + p
            if h >= H:
                continue
            o = h * BR + br
            s = (one - beta[h, br]) * (one - alpha[h])
            sc2[p, m] = s
            sc28[p, m] = s * np.float32(1.0 / PSCALE)
            b2[p, m] = s * b_in[o]
            bt[p, m] = beta[h, br]
            wrow = np.zeros(DP, np.float32)
            wrow[:D_IN] = W_in[o]
            wt[:, :, m * 128 + p] = wrow.reshape(NK, 128)
    wh = wt.astype(np.float16).astype(np.float32)
    wl = wt - wh
    # w splits: 0=wh1, 1=wh2, 2=wa, 3=wb (scaled, stored fp8)
    wh1 = fp8q(wh * np.float32(SWH))
    wh2 = fp8q(wh * np.float32(SWH) - wh1.astype(np.float32))
    wa = fp8q(wl * np.float32(SWL))
    wb = fp8q(wl * np.float32(SWL) - wa.astype(np.float32))
    wsplit_all = [wh1, wh2, wa, wb]
    w8 = np.stack([wsplit_all[i] for i in sorted({p[0] for p in prods})])

    atile = np.zeros((128, NF), np.float32)
    for j in range(NJ):
        for p in range(128):
            h = j * 128 + p
            if h >= H:
                continue
            atile[p, j * BL:(j + 1) * BL] = alpha[h]
    woutT = np.zeros((256, D_OUT), np.float32)
    woutT[:H, :] = 0.5 * W_out.T
    teff = np.float32(T_ - chunks[-1])
    bout = (b_out + 0.5 * teff * W_out.sum(axis=1)).reshape(D_OUT, 1)

    shared = dict(wt=(wt * np.float32(PSCALE)).astype(np.float16),
                  w8=w8.view(np.uint8),
                  sc2=sc2, sc28=sc28, b2=b2, bt=bt, atile=atile,
                  woutT=_f32(woutT), bout=_f32(bout))

    percore = []
    for core in range(NCORES):
        xl_ = x[core * BL:(core + 1) * BL, :T_, :]        # (BL,T,D_IN)
        xp_ = np.zeros((BL, T_, DP), np.float32)
        xp_[:, :, :D_IN] = xl_
        xh = xp_.astype(np.float16).astype(np.float32)
        xlo = xp_ - xh
        # x splits: 0=xa, 1=xb, 2=xh1, 3=xh2
        xa = fp8q(xlo * np.float32(SXA))
        xsplit = {0: xa}
        if 1 in xvars:
            xsplit[1] = fp8q(xlo * np.float32(SXA) - xa.astype(np.float32))
        xh1 = fp8q(xh * np.float32(SXH))
        xsplit[2] = xh1
        if 3 in xvars:
            xsplit[3] = fp8q(xh * np.float32(SXH) - xh1.astype(np.float32))

        FT16 = sum(NK * BG * cc for cc in chunks)
        FT8 = sum(NK * NXV * BG * cc for cc in chunks)
        xt = np.zeros((NG, 128, FT16), np.float16)
        x8 = np.zeros((NG, 128, FT8), E4M3)
        colo16 = 0
        colo8 = 0
        t0 = 0
        xh16 = xp_.astype(np.float16)
        for cc in chunks:
            for g in range(NG):
                sub = xh16[g * BG:(g + 1) * BG, t0:t0 + cc, :]   # (BG,cc,DP)
                sg = sub.reshape(BG, cc, NK, 128).transpose(3, 2, 0, 1)
                xt[g, :, colo16:colo16 + NK * BG * cc] = sg.reshape(128, -1)
                # x8 layout: (k, v, b, t)
                blk = np.empty((128, NK, NXV, BG, cc), E4M3)
                for vi, v in enumerate(xvars):
                    sv = xsplit[v][g * BG:(g + 1) * BG, t0:t0 + cc, :]
                    blk[:, :, vi] = sv.reshape(BG, cc, NK, 128).transpose(
                        3, 2, 0, 1)
                x8[g, :, colo8:colo8 + NK * NXV * BG * cc] = blk.reshape(128, -1)
            colo16 += NK * BG * cc
            colo8 += NK * NXV * BG * cc
            t0 += cc
        percore.append(dict(xt=xt, x8=x8.view(np.uint8)))
    uni = float(alpha[0]) if np.all(alpha == alpha[0]) else None
    return shared, percore, uni


TRACE = False          # set by test harness for profiling runs
LAST_RESULT = None
NPROD = 2


def kernel(x, W_in, b_in, tau_n, tau_m, W_out, b_out):
    global LAST_RESULT
    from concourse.bass_utils import run_bass_kernel_spmd

    shared, percore, uni = _prep_host(x, W_in, b_in, tau_n, tau_m, W_out,
                                      b_out, T, CHUNKS, NPROD)
    nc = _build(T, CHUNKS, NPROD, alpha_uniform_val=uni)
    tok = np.zeros((1, 16), np.float32)
    in_maps = [dict(shared, tok=tok, **percore[core])
               for core in range(NCORES)]
    res = run_bass_kernel_spmd(nc, in_maps, core_ids=list(range(NCORES)),
                               trace=TRACE)
    LAST_RESULT = res
    out = np.empty((B, D_OUT), np.float32)
    for core in range(NCORES):
        out[core * BL:(core + 1) * BL, :] = res.results[core]["out"].T
    return out


# revision 21
# speedup vs baseline: 1.0622x; 1.0049x over previous
"""Trainium2 Bass kernel for the dendritic-branch spiking FNN (DH_SFNN).

Model (per reference):
  branch_in = x @ W_in.T + b_in                  # (B,T,H*BR)
  per t:  i_d = beta*i_d + (1-beta)*branch_in_t  # beta = sigmoid(tau_n), (H,BR)
          v   = alpha*v + (1-alpha)*i_d.sum(br)  # alpha = sigmoid(tau_m), (H,)
          spike = (v >= 1); v -= spike; counts += spike
  out = counts @ W_out.T + b_out                 # (B,D_OUT)

Strategy: data-parallel over batch across 8 cores (32 rows each). Per core,
T=500 is processed in chunks pipelined across engines.

GEMM: fp16 "hh" pass (W_hi @ x_hi, 6 k-tiles) into PSUM P0, plus fp8e4m3
DoubleRow correction products into PSUM P1. The corrections recover the
fp16-split residuals (W_lo @ x_hi + W_hi @ x_lo) from power-of-2-scaled fp8
operands; DoubleRow runs fp8 matmuls at 0.5 cycles/row with 256-deep
contraction, so each correction product costs 1/4 of an fp16 pass. The W
operands are UNSCALED (uniform row magnitudes) -- folding the per-row
(1-beta)(1-alpha) scale into W pushes the fp8 splits into subnormal flush and
fails the accuracy gate; instead sc2 stays in the Act epilogue:
  Act#1: u0 = sc2*P0 + b2   (baseline epilogue)
  Act#2: c  = (sc2*2^-17)*P1   (per-partition scale AP)
  Pool:  u  = u0 + c
Host emulation: 2-product corrections reproduce the fp32 reference's spike
train exactly on this data (rel 1e-7, same as the fp16 3-pass baseline).

Engines:
  PE  : hh fp16 matmuls + fp8 DR corrections + readout
  Act : PSUM epilogues (two per (m,g)), Sign() spike-compare on hist
  Pool: epilogue combine adds, branch-sum adds
  DVE : IIR scans, carry handling, serial per-timestep spike loop,
        spike-count time reduction
"""

import sys

if "/opt/trn_rl_repo" not in sys.path:
    sys.path.insert(0, "/opt/trn_rl_repo")

from contextlib import ExitStack

import numpy as np
import ml_dtypes

import concourse.bass as bass
import concourse.mybir as mybir
import concourse.tile as tile
from concourse import bacc

B, T, D_IN, H, BR, D_OUT = 256, 500, 700, 200, 2, 35
NCORES = 8
BL = B // NCORES          # local batch = 32
NK = 6                    # k-tiles; D_IN padded 700 -> 768 so every tile is 128
DP = NK * 128             # padded contraction dim (768)
M = 4                     # m-tiles, m=(br,j): o'' = m*128 + p, h = (m%2)*128+p
OP = M * 128              # padded output rows (512)
NJ = 2                    # h groups (j=0: h<128, j=1: h 128..199)
NF = NJ * BL              # spike-loop state columns (64)
BG = 8                    # batches per matmul n-group
NG = BL // BG             # 4 n-groups

CHUNKS = (50,) * 9 + (25, 15, 10)     # sum = T; shrinking tail chunks

# fp8 correction scheme: NPROD products (wvar, xvar); all products land at
# scale PSCALE = 2^16 in the SAME PSUM bank as the fp16 hh pass, whose weights
# are pre-scaled by 2^16 (exact power-of-2 in fp16; max |W|*2^16 ~ 13K < 65504).
# The single Act epilogue applies sc2*2^-16.
SXA, SWH = 2.0 ** 11, 2.0 ** 5        # x_lo*SXA, W_hi*SWH
SXH, SWL = 2.0 ** 4, 2.0 ** 12        # x_hi*SXH, W_lo*SWL
PSCALE = 2.0 ** 16
E4M3 = ml_dtypes.float8_e4m3

# product tables by nprod: (w split index, x split index)
# w splits: 0=wh1, 1=wh2, 2=wa, 3=wb ; x splits: 0=xa, 1=xb, 2=xh1, 3=xh2
PRODUCTS = {
    2: [(0, 0), (2, 2)],
    6: [(0, 0), (0, 1), (1, 0), (2, 2), (3, 2), (2, 3)],
}
XVARS = {2: [0, 2], 6: [0, 1, 2, 3]}  # distinct x splits shipped


def _f32(a):
    return np.ascontiguousarray(a, dtype=np.float32)


def _build(T_, chunks, nprod, alpha_uniform_val=None):
    chunks = tuple(chunks)
    NCH = len(chunks)
    assert sum(chunks) == T_
    C0 = max(chunks)
    sizes = sorted(set(chunks))
    prods = PRODUCTS[nprod]
    xvars = XVARS[nprod]
    NXV = len(xvars)
    xv_pos = {v: i for i, v in enumerate(xvars)}   # x split -> slot in x8
    wlist = sorted({p[0] for p in prods})          # distinct w splits shipped
    w_pos = {v: i for i, v in enumerate(wlist)}
    NW = len(wlist)

    # x columns per (chunk, group) and offsets in the flat tensors
    xoff16 = np.cumsum([0] + [NK * BG * cc for cc in chunks]).tolist()
    FT16 = xoff16[-1]
    xoff8 = np.cumsum([0] + [NK * NXV * BG * cc for cc in chunks]).tolist()
    FT8 = xoff8[-1]
    # d0 blocks per distinct chunk size
    soff = {}
    off = 0
    for s in sizes:
        soff[s] = off
        off += M * BL * s
    SD = off

    fp32 = mybir.dt.float32
    fp16 = mybir.dt.float16
    fp8 = mybir.dt.float8e4
    AF = mybir.ActivationFunctionType
    AL = mybir.AluOpType
    PM = mybir.MatmulPerfMode

    nc = bacc.Bacc("TRN2", target_bir_lowering=False, debug=False,
                   num_devices=NCORES)

    xt_d = nc.dram_tensor("xt", [NG, 128, FT16], fp16, kind="ExternalInput")
    x8_d = nc.dram_tensor("x8", [NG, 128, FT8], fp8, kind="ExternalInput")
    wt_d = nc.dram_tensor("wt", [NK, 128, OP], fp16, kind="ExternalInput")
    w8_d = nc.dram_tensor("w8", [NW, NK, 128, OP], fp8, kind="ExternalInput")
    sc2_d = nc.dram_tensor("sc2", [128, M], fp32, kind="ExternalInput")
    sc28_d = nc.dram_tensor("sc28", [128, M], fp32, kind="ExternalInput")
    b2_d = nc.dram_tensor("b2", [128, M], fp32, kind="ExternalInput")
    bt_d = nc.dram_tensor("bt", [128, M], fp32, kind="ExternalInput")
    atile_d = nc.dram_tensor("atile", [128, NF], fp32, kind="ExternalInput")
    woutT_d = nc.dram_tensor("woutT", [2 * 128, D_OUT], fp32, kind="ExternalInput")
    bout_d = nc.dram_tensor("bout", [D_OUT, 1], fp32, kind="ExternalInput")

    out_d = nc.dram_tensor("out", [D_OUT, BL], fp32, kind="ExternalOutput")
    tok_d = nc.dram_tensor("tok", [1, 16], fp32, kind="ExternalInput")
    tok_o = nc.dram_tensor("tok_out", [1, 16], fp32, kind="ExternalOutput")

    with tile.TileContext(nc) as tc, ExitStack() as ctx:
        const = ctx.enter_context(tc.tile_pool(name="const", bufs=1))
        st = ctx.enter_context(tc.tile_pool(name="state", bufs=1))
        up = ctx.enter_context(tc.tile_pool(name="up", bufs=2))
        wp = ctx.enter_context(tc.tile_pool(name="wph", bufs=2))
        hp = ctx.enter_context(tc.tile_pool(name="hp", bufs=2))
        xp = ctx.enter_context(tc.tile_pool(name="xin", bufs=3))
        ps = ctx.enter_context(tc.tile_pool(name="psum", bufs=2, space="PSUM"))
        pso = ctx.enter_context(tc.tile_pool(name="psout", bufs=1, space="PSUM"))
        scr = ctx.enter_context(tc.tile_pool(name="scr", bufs=2))

        w_sb = const.tile([128, NK * OP], fp16, tag="wsb")
        nc.sync.dma_start(
            w_sb[:].rearrange("p (k o) -> p k o", k=NK),
            wt_d.ap().rearrange("k p o -> p k o"))
        w8_sb = const.tile([128, NW * NK * OP], fp8, tag="w8sb")
        nc.scalar.dma_start(
            w8_sb[:].rearrange("p (q k o) -> p q k o", q=NW, k=NK),
            w8_d.ap().rearrange("q k p o -> p q k o"))
        sc2 = const.tile([128, M], fp32)
        nc.sync.dma_start(sc2[:], sc2_d.ap())
        sc28 = const.tile([128, M], fp32)
        nc.sync.dma_start(sc28[:], sc28_d.ap())
        b2 = const.tile([128, M], fp32)
        nc.sync.dma_start(b2[:], b2_d.ap())
        bt = const.tile([128, M], fp32)
        nc.scalar.dma_start(bt[:], bt_d.ap())
        atile = const.tile([128, NF], fp32)
        nc.scalar.dma_start(atile[:], atile_d.ap())
        # d0 (scan multipliers: beta everywhere, 0 at each batch's t=0)
        d0_sb = const.tile([128, SD], fp32)
        nc.vector.memset(d0_sb[:], 0.0)
        for si, s in enumerate(sizes):
            for m in range(M):
                blk = d0_sb[:, soff[s] + m * BL * s:
                            soff[s] + (m + 1) * BL * s]
                nc.scalar.activation(blk, blk, AF.Identity,
                                     bias=bt[:, m:m + 1], scale=0.0)
                nc.vector.memset(
                    blk.rearrange("p (b c) -> p b c", c=s)[:, :, 0], 0.0)
        woutT_sb = const.tile([128, 2 * D_OUT], fp32)
        nc.scalar.dma_start(woutT_sb[:, 0:D_OUT], woutT_d.ap()[0:128])
        nc.scalar.dma_start(woutT_sb[:, D_OUT:2 * D_OUT], woutT_d.ap()[128:256])
        bout_sb = const.tile([D_OUT, 1], fp32)
        nc.scalar.dma_start(bout_sb[:], bout_d.ap())
        w8v = w8_sb[:].rearrange("p (q k o) -> p q k o", q=NW, k=NK)

        def count_ops(hist, cc, counts):
            """DVE ops for the spike-count of a finished chunk: is_le turns
            hist's negated pre-reset potentials into {0,1} spikes, then a
            contiguous fold-tree sums over time. Chopped into pieces that
            fill the FIRST half of the next spike chain (hist is ready at
            chunk start, unlike the scan pieces)."""
            ops = []
            npc = 4 if cc >= 20 else 1
            step = (cc + npc - 1) // npc
            for t0 in range(0, cc, step):
                t1 = min(t0 + step, cc)
                def isle(t0=t0, t1=t1):
                    nc.vector.tensor_scalar(
                        hist[:, t0 * NF:t1 * NF], hist[:, t0 * NF:t1 * NF],
                        -1.0, None, AL.is_le)
                ops.append(isle)
            n = cc
            while n > 1:
                h = n // 2
                spl = 2 if h * NF > 1024 else 1
                bs = (h + spl - 1) // spl
                for b0 in range(0, h, bs):
                    b1 = min(b0 + bs, h)
                    def fold(b0=b0, b1=b1, h=h):
                        nc.vector.tensor_tensor(
                            hist[:, b0 * NF:b1 * NF],
                            hist[:, b0 * NF:b1 * NF],
                            hist[:, (h + b0) * NF:(h + b1) * NF], AL.add)
                    ops.append(fold)
                if n % 2:
                    def strag(n=n):
                        nc.vector.tensor_tensor(
                            hist[:, 0:NF], hist[:, 0:NF],
                            hist[:, (n - 1) * NF:n * NF], AL.add)
                    ops.append(strag)
                n = h
            def final():
                nc.vector.tensor_tensor(counts[:], counts[:], hist[:, 0:NF],
                                        AL.add)
            ops.append(final)
            return ops

        MORDER = (0, 2, 1, 3)   # m emission order; pairs (0,2),(1,3) feed j0,j1
        SP = 256                # scan piece length (columns)

        def emit_gemm(c, CC, u):
            """GEMM for chunk c into tile u. n-groups processed in pairs
            sharing each weight load; m-tiles in MORDER so the branch-sum
            inputs complete early-first."""
            NNc = BG * CC
            ndr = len(prods) * (NK // 2)
            for gp in range(NG // 2):
                xs16 = []
                xs8 = []
                for gi in range(2):
                    g = 2 * gp + gi
                    x16 = xp.tile([128, NK * BG * C0], fp16, tag="x16")
                    nc.sync.dma_start(
                        x16[:, 0:NK * NNc],
                        xt_d.ap()[g][:, xoff16[c]:xoff16[c + 1]])
                    x8 = xp.tile([128, NK * NXV * BG * C0], fp8, tag="x8")
                    nc.sync.dma_start(
                        x8[:, 0:NK * NXV * NNc],
                        x8_d.ap()[g][:, xoff8[c]:xoff8[c + 1]])
                    xs16.append(x16)
                    xs8.append(x8[:, 0:NK * NXV * NNc].rearrange(
                        "p (k v n) -> p k v n", k=NK, v=NXV))
                for m in MORDER:
                    pts = [ps.tile([128, NNc], fp32, tag=f"pt{gi}",
                                   name=f"pt{gi}") for gi in range(2)]
                    for k in range(NK):
                        wap = w_sb[:, k * OP + m * 128:
                                   k * OP + (m + 1) * 128]
                        for gi in range(2):
                            nc.tensor.matmul(
                                pts[gi][:], wap,
                                xs16[gi][:, k * NNc:(k + 1) * NNc],
                                start=(k == 0), stop=False)
                    i = 0
                    for (wv, xv) in prods:
                        for j in range(NK // 2):
                            w8ap = w8v[:, w_pos[wv], 2 * j:2 * j + 2,
                                       m * 128:(m + 1) * 128]
                            for gi in range(2):
                                nc.tensor.matmul(
                                    pts[gi][:], w8ap,
                                    xs8[gi][:, 2 * j:2 * j + 2, xv_pos[xv]],
                                    start=False, stop=(i == ndr - 1),
                                    perf_mode=PM.DoubleRow)
                            i += 1
                    for gi in range(2):
                        g = 2 * gp + gi
                        nc.scalar.activation(
                            u[:, m * BL * C0 + g * NNc:
                              m * BL * C0 + (g + 1) * NNc],
                            pts[gi][:], AF.Identity,
                            bias=b2[:, m:m + 1], scale=sc28[:, m:m + 1])

        def scan_ops(c, CC, u, carry, sp=None):
            """DVE ops for chunk c's dendrite IIR, chopped into batch-aligned
            pieces. At every batch start d0 is 0 (or handled by the Pool
            carry-inject for c>0), so pieces take initial=0.0 and carry NO
            dependency on each other -- they interleave freely into the
            previous chunk's spike chain."""
            ops = []
            L = BL * CC
            if sp is None:
                sp = 4 * CC             # 4 batches per piece
            assert sp % CC == 0
            for m in MORDER:
                base = m * BL * C0
                um = u[:, base:base + L]
                um3 = um.rearrange("p (b c) -> p b c", c=CC)
                d0c = soff[CC] + m * BL * CC
                if c > 0:
                    # inject beta*carry into each batch's first column on
                    # Pool (carry was pre-scaled by beta at copy time)
                    def carry_add(m=m, um3=um3):
                        nc.gpsimd.tensor_tensor(
                            um3[:, :, 0], um3[:, :, 0],
                            carry[:, m * BL:(m + 1) * BL], AL.add)
                    ops.append(carry_add)
                p0 = 0
                while p0 < L:
                    p1 = min(p0 + sp, L)
                    def piece(p0=p0, p1=p1, um=um, d0c=d0c):
                        nc.vector.tensor_tensor_scan(
                            um[:, p0:p1], d0_sb[:, d0c + p0:d0c + p1],
                            um[:, p0:p1], 0.0, AL.mult, AL.add)
                    ops.append(piece)
                    p0 = p1
                if c < NCH - 1:
                    # carry := beta * i_d[last col]  (pre-scaled for inject)
                    def carry_copy(m=m, um3=um3, CC=CC):
                        nc.vector.tensor_scalar(
                            carry[:, m * BL:(m + 1) * BL], um3[:, :, CC - 1],
                            bt[:, m:m + 1], None, AL.mult)
                    ops.append(carry_copy)
            return ops

        def emit_adds(CC, u, wti, halves):
            """Branch sums w_j = i'_d[j] + i'_d[2+j] on Pool, optionally in
            batch-halves so the early halves start before all scans finish."""
            wre = wti[:, 0:CC * NF].rearrange("p (c j b) -> p b j c",
                                              j=NJ, b=BL)
            spans = [(0, BL // 2), (BL // 2, BL)] if halves else [(0, BL)]
            for j in range(NJ):
                ua = u[:, j * BL * C0:j * BL * C0 + BL * CC].rearrange(
                    "p (b c) -> p b c", c=CC)
                ub = u[:, (2 + j) * BL * C0:(2 + j) * BL * C0 + BL * CC
                       ].rearrange("p (b c) -> p b c", c=CC)
                for (b0, b1) in spans:
                    nc.gpsimd.tensor_tensor(
                        wre[:, b0:b1, j, :], ua[:, b0:b1], ub[:, b0:b1],
                        AL.add)

        def body_once():
            vst = st.tile([128, NF], fp32, tag="vst")  # negated potential
            counts = st.tile([128, NF], fp32, tag="cnt")
            carry = st.tile([128, M * BL], fp32, tag="carry")
            nc.vector.memset(vst[:], 0.0)
            nc.vector.memset(counts[:], 0.0)

            # prologue: chunk 0's GEMM + scans + branch sums up front
            u_cur = up.tile([128, M * BL * C0], fp32, tag="u", name="u0")
            emit_gemm(0, chunks[0], u_cur)
            for op in scan_ops(0, chunks[0], u_cur, carry, sp=BL * chunks[0]):
                op()
            wti_cur = wp.tile([128, C0 * NF], fp32, tag="wti", name="wti0")
            emit_adds(chunks[0], u_cur, wti_cur, halves=False)

            prev = None     # (hist, chunk_len) of previous chunk
            for c, CC in enumerate(chunks):
                # next chunk's GEMM + its scan pieces (interleaved below)
                if c + 1 < NCH:
                    CCn = chunks[c + 1]
                    u_next = up.tile([128, M * BL * C0], fp32, tag="u",
                                     name="un")
                    emit_gemm(c + 1, CCn, u_next)
                    pieces = scan_ops(c + 1, CCn, u_next, carry)
                else:
                    u_next = None
                    pieces = []
                early = (count_ops(prev[0], prev[1], counts)
                         if prev is not None else [])

                hist = hp.tile([128, C0 * NF], fp32, tag="hist")

                # -- spike loop (negated state: vt = -v). The previous
                # chunk's count pieces fill the first half of the chain's
                # semaphore gaps; the next chunk's scan pieces (whose inputs
                # stream out of the concurrent GEMM) fill the second half --
                slots = 2 * CC
                start = slots // 2
                n_e = len(early)
                n_l = len(pieces)
                ie = 0
                il = 0
                slot = 0

                def drain():
                    nonlocal ie, il
                    while ie < n_e and ie * start <= slot * n_e:
                        early[ie]()
                        ie += 1
                    while (il < n_l and slot > start and
                           il * (slots - start) <= (slot - start) * n_l):
                        pieces[il]()
                        il += 1

                for t in range(CC):
                    tA = hist[:, t * NF:(t + 1) * NF]   # pre-reset vt' kept
                    wt_t = wti_cur[:, t * NF:(t + 1) * NF]
                    if alpha_uniform_val is not None:
                        nc.vector.scalar_tensor_tensor(
                            tA, vst[:], float(alpha_uniform_val),
                            wt_t, AL.mult, AL.subtract)
                    else:
                        nc.vector.tensor_tensor(tA, vst[:], atile[:], AL.mult)
                        nc.vector.tensor_tensor(tA, tA, wt_t, AL.subtract)
                    slot += 1
                    drain()
                    nc.vector.scalar_tensor_tensor(
                        vst[:], tA, -1.0, tA, AL.is_le, AL.add)
                    slot += 1
                    drain()
                while ie < n_e:
                    early[ie]()
                    ie += 1
                while il < n_l:
                    pieces[il]()
                    il += 1

                if c + 1 < NCH:
                    wti_next = wp.tile([128, C0 * NF], fp32, tag="wti",
                                       name="wtin")
                    emit_adds(chunks[c + 1], u_next, wti_next, halves=True)
                    u_cur, wti_cur = u_next, wti_next
                prev = (hist, CC)

            # final chunk's count inline
            for op in count_ops(prev[0], prev[1], counts):
                op()

            # -- readout --
            po = pso.tile([D_OUT, BL], fp32, tag="po")
            nc.tensor.matmul(po[:], woutT_sb[:, 0:D_OUT], counts[:, 0:BL],
                             start=True, stop=False)
            nc.tensor.matmul(po[:], woutT_sb[0:H - 128, D_OUT:2 * D_OUT],
                             counts[0:H - 128, BL:2 * BL], start=False,
                             stop=True)
            out_sb = scr.tile([D_OUT, BL], fp32, tag="osb")
            nc.scalar.activation(out_sb[:], po[:], AF.Identity,
                                 bias=bout_sb[:, 0:1], scale=1.0)
            nc.sync.dma_start(out_d.ap(), out_sb[:])

        body_once()
        tok_sb = scr.tile([1, 16], fp32, tag="tok")
        nc.sync.dma_start(tok_sb[:], tok_d.ap())
        nc.sync.dma_start(tok_o.ap(), tok_sb[:])

    nc.compile()
    return nc


def _prep_host(x, W_in, b_in, tau_n, tau_m, W_out, b_out, T_, chunks, nprod):
    """Host-side constant prep. Returns (shared_inputs, per_core_inputs, alpha_uni)."""
    x = _f32(x); W_in = _f32(W_in); b_in = _f32(b_in)
    tau_n = _f32(tau_n); tau_m = _f32(tau_m)
    W_out = _f32(W_out); b_out = _f32(b_out)
    chunks = tuple(chunks)
    assert sum(chunks) == T_
    prods = PRODUCTS[nprod]
    xvars = XVARS[nprod]
    NXV = len(xvars)
    NW = len({p[0] for p in prods})

    beta = _f32(1.0 / (1.0 + np.exp(-tau_n.astype(np.float64))))   # (H,BR)
    alpha = _f32(1.0 / (1.0 + np.exp(-tau_m.astype(np.float64))))  # (H,)
    one = np.float32(1.0)

    def fp8q(a):
        return np.ascontiguousarray(a, dtype=np.float32).astype(E4M3)

    # m-tile map: m=(br,j) -> rows p: h = (m%2)*128+p, o = h*BR + br
    wt = np.zeros((NK, 128, OP), np.float32)
    sc2 = np.zeros((128, M), np.float32)
    sc28 = np.zeros((128, M), np.float32)
    b2 = np.zeros((128, M), np.float32)
    bt = np.zeros((128, M), np.float32)
    for m in range(M):
        br, j = m // 2, m % 2
        for p in range(128):
            h = j * 128 + p
            if h >= H:
                continue
            o = h * BR + br
            s = (one - beta[h, br]) * (one - alpha[h])
            sc2[p, m] = s
            sc28[p, m] = s * np.float32(1.0 / PSCALE)
            b2[p, m] = s * b_in[o]
            bt[p, m] = beta[h, br]
            wrow = np.zeros(DP, np.float32)
            wrow[:D_IN] = W_in[o]
            wt[:, :, m * 128 + p] = wrow.reshape(NK, 128)
    wh = wt.astype(np.float16).astype(np.float32)
    wl = wt - wh
    # w splits: 0=wh1, 1=wh2, 2=wa, 3=wb (scaled, stored fp8)
    wh1 = fp8q(wh * np.float32(SWH))
    wh2 = fp8q(wh * np.float32(SWH) - wh1.astype(np.float32))
    wa = fp8q(wl * np.float32(SWL))
    wb = fp8q(wl * np.float32(SWL) - wa.astype(np.float32))
    wsplit_all = [wh1, wh2, wa, wb]
    w8 = np.stack([wsplit_all[i] for i in sorted({p[0] for p in prods})])

    atile = np.zeros((128, NF), np.float32)
    for j in range(NJ):
        for p in range(128):
            h = j * 128 + p
            if h >= H:
                continue
            atile[p, j * BL:(j + 1) * BL] = alpha[h]
    woutT = np.zeros((256, D_OUT), np.float32)
    woutT[:H, :] = W_out.T
    bout = b_out.reshape(D_OUT, 1).copy()

    shared = dict(wt=(wt * np.float32(PSCALE)).astype(np.float16),
                  w8=w8.view(np.uint8),
                  sc2=sc2, sc28=sc28, b2=b2, bt=bt, atile=atile,
                  woutT=_f32(woutT), bout=_f32(bout))

    percore = []
    for core in range(NCORES):
        xl_ = x[core * BL:(core + 1) * BL, :T_, :]        # (BL,T,D_IN)
        xp_ = np.zeros((BL, T_, DP), np.float32)
        xp_[:, :, :D_IN] = xl_
        xh = xp_.astype(np.float16).astype(np.float32)
        xlo = xp_ - xh
        # x splits: 0=xa, 1=xb, 2=xh1, 3=xh2
        xa = fp8q(xlo * np.float32(SXA))
        xsplit = {0: xa}
        if 1 in xvars:
            xsplit[1] = fp8q(xlo * np.float32(SXA) - xa.astype(np.float32))
        xh1 = fp8q(xh * np.float32(SXH))
        xsplit[2] = xh1
        if 3 in xvars:
            xsplit[3] = fp8q(xh * np.float32(SXH) - xh1.astype(np.float32))

        FT16 = sum(NK * BG * cc for cc in chunks)
        FT8 = sum(NK * NXV * BG * cc for cc in chunks)
        xt = np.zeros((NG, 128, FT16), np.float16)
        x8 = np.zeros((NG, 128, FT8), E4M3)
        colo16 = 0
        colo8 = 0
        t0 = 0
        xh16 = xp_.astype(np.float16)
        for cc in chunks:
            for g in range(NG):
                sub = xh16[g * BG:(g + 1) * BG, t0:t0 + cc, :]   # (BG,cc,DP)
                sg = sub.reshape(BG, cc, NK, 128).transpose(3, 2, 0, 1)
                xt[g, :, colo16:colo16 + NK * BG * cc] = sg.reshape(128, -1)
                # x8 layout: (k, v, b, t)
                blk = np.empty((128, NK, NXV, BG, cc), E4M3)
                for vi, v in enumerate(xvars):
                    sv = xsplit[v][g * BG:(g + 1) * BG, t0:t0 + cc, :]
                    blk[:, :, vi] = sv.reshape(BG, cc, NK, 128).transpose(
                        3, 2, 0, 1)
                x8[g, :, colo8:colo8 + NK * NXV * BG * cc] = blk.reshape(128, -1)
            colo16 += NK * BG * cc
            colo8 += NK * NXV * BG * cc
            t0 += cc
        percore.append(dict(xt=xt, x8=x8.view(np.uint8)))
    uni = float(alpha[0]) if np.all(alpha == alpha[0]) else None
    return shared, percore, uni


TRACE = False          # set by test harness for profiling runs
LAST_RESULT = None
NPROD = 2


def kernel(x, W_in, b_in, tau_n, tau_m, W_out, b_out):
    global LAST_RESULT
    from concourse.bass_utils import run_bass_kernel_spmd

    shared, percore, uni = _prep_host(x, W_in, b_in, tau_n, tau_m, W_out,
                                      b_out, T, CHUNKS, NPROD)
    nc = _build(T, CHUNKS, NPROD, alpha_uniform_val=uni)
    tok = np.zeros((1, 16), np.float32)
    in_maps = [dict(shared, tok=tok, **percore[core])
               for core in range(NCORES)]
    res = run_bass_kernel_spmd(nc, in_maps, core_ids=list(range(NCORES)),
                               trace=TRACE)
    LAST_RESULT = res
    out = np.empty((B, D_OUT), np.float32)
    for core in range(NCORES):
        out[core * BL:(core + 1) * BL, :] = res.results[core]["out"].T
    return out
